# revision 47
# baseline (speedup 1.0000x reference)
"""Trainium2 Bass kernel for nn_AttnBlock_Spatio_Temporal (B=4,T=5,C=512,H=W=32).

Distribution: 8 cores = (video b in 0..3) x (pixel-half h in 0..1); host rolls
the HW axis per core so its own 512 pixels come first. All heavy matmuls run
in fp8e4 DoubleRow (K=256 per MM). Spatial attention computes scores
TRANSPOSED (keys on partitions) so no attention transpose is needed; softmax
normalization is deferred through the v- and wo-matmuls and applied once per
output pixel (PE broadcast of 1/den). GroupNorm_t cross-half stats use two
batched pair-AllReduces (frames 0-3 in one [128,2] op, frame 4 in [32,2]).

Channel-major layout: channel c lives at (partition p, block j), c = 4p + j.
Weight matrices are host-permuted (columns) and pre-scaled x16 into fp8.
"""
import numpy as np

B, T, C, HW = 4, 5, 512, 1024
G = 32
EPS = 1e-6
P = 128
CB = C // P          # 4 channel blocks
HALF = HW // 2       # 512 own pixels
KB = HW // P         # 8 key-pixel blocks
QB = HALF // P       # 4 query/pixel blocks
SCALE = float(C) ** -0.5
INV_CNT = 1.0 / 16384.0   # per-group element count (16ch * 1024px)

_CACHE = {}


def _build():
    import concourse.bacc as bacc
    import concourse.tile as tile
    import concourse.mybir as mybir

    f32 = mybir.dt.float32
    bf16 = mybir.dt.bfloat16
    fp8 = mybir.dt.float8e4
    MULT = mybir.AluOpType.mult
    ADD = mybir.AluOpType.add
    SUB = mybir.AluOpType.subtract
    AF = mybir.ActivationFunctionType
    AX = mybir.AxisListType
    DR = mybir.MatmulPerfMode.DoubleRow

    nc = bacc.Bacc("TRN2", target_bir_lowering=False, debug=False, num_devices=8)

    x_d = nc.dram_tensor("x", [T, C, HW], f32, kind="ExternalInput").ap()
    fp8_w = ["wq", "wk", "wv", "wo", "wqt", "wkt", "wvt"]
    w_d = {nm: nc.dram_tensor(nm + "T", [C, C], fp8, kind="ExternalInput").ap()
           for nm in fp8_w}
    wot_d = nc.dram_tensor("wotT", [C, C], fp8, kind="ExternalInput").ap()
    g_d = {nm: nc.dram_tensor(nm, [C], f32, kind="ExternalInput").ap()
           for nm in ["gamma_s", "beta_s16", "gamma_t", "beta_t16"]}
    sel_d = nc.dram_tensor("sel4", [P, G], f32, kind="ExternalInput").ap()
    bc16_d = nc.dram_tensor("bcast16", [G, P], f32, kind="ExternalInput").ap()
    bc16f_d = nc.dram_tensor("bcast16f", [P, 5, P], f32,
                             kind="ExternalInput").ap()
    ind5_d = nc.dram_tensor("ind5", [32, 5], bf16, kind="ExternalInput").ap()
    sel25_d = nc.dram_tensor("sel25", [P, 25, 25], bf16,
                             kind="ExternalInput").ap()
    out_d = nc.dram_tensor("out", [T, C, HALF], f32, kind="ExternalOutput").ap()

    def cpart(ap_1d):  # [C] dram -> [128, CB] tile order (c = 4p + j)
        return ap_1d.rearrange("(p j) -> p j", p=P)

    with tile.TileContext(nc) as tc, \
         nc.allow_low_precision("fp8/bf16 pipeline by design"):
        with tc.tile_pool(name="consts", bufs=1) as consts, \
             tc.tile_pool(name="stat", bufs=2) as stat, \
             tc.tile_pool(name="spatio_p", bufs=T) as spatio_p, \
             tc.tile_pool(name="gnt_p", bufs=T) as gnt_p, \
             tc.tile_pool(name="psA", bufs=3, space="PSUM") as psA, \
             tc.tile_pool(name="psS", bufs=2, space="PSUM") as psS, \
             tc.tile_pool(name="dram", bufs=4, space="DRAM") as dram:

            # ---------------- constants ----------------
            spat_cm = tc.tile_pool(name="spat", bufs=1)
            spat = spat_cm.__enter__()
            nc.sync.dma_start(
                out=(xf0 := spat.tile([P, CB, HW], f32, tag="xf", name="xf0",
                                      bufs=2)),
                in_=x_d[0].rearrange("(p j) hw -> p j hw", p=P))
            w_sb = {}
            for nm in ["wk", "wq", "wv"]:
                w_sb[nm] = consts.tile([P, CB, C], fp8, tag="w_" + nm,
                                       name="w_" + nm)
                nc.sync.dma_start(
                    out=w_sb[nm],
                    in_=w_d[nm].rearrange("(p kc) co -> p kc co", p=P))
            gam_sb = {}
            for nm in ["gamma_s", "beta_s16", "gamma_t", "beta_t16"]:
                gam_sb[nm] = consts.tile([P, CB], f32, tag="g_" + nm,
                                         name="g_" + nm)
                nc.sync.dma_start(out=gam_sb[nm], in_=cpart(g_d[nm]))
            sel4 = consts.tile([P, G], f32, tag="sel4", name="sel4")
            nc.sync.dma_start(out=sel4, in_=sel_d)
            bc16 = consts.tile([G, P], f32, tag="bc16", name="bc16")
            nc.sync.dma_start(out=bc16, in_=bc16_d)
            bc16f = consts.tile([P, 5, P], f32, tag="bc16f", name="bc16f")
            nc.sync.dma_start(out=bc16f, in_=bc16f_d)
            eps128 = consts.tile([P, 1], f32, tag="eps", name="eps128")
            nc.vector.memset(eps128, EPS)
            ones1f = consts.tile([P, 1], fp8, tag="ones1f", name="ones1f")
            nc.vector.memset(ones1f, 1.0)
            ind5 = consts.tile([32, 5], bf16, tag="ind5", name="ind5")
            nc.sync.dma_start(out=ind5, in_=ind5_d)
            sel25 = consts.tile([P, 25, 25], bf16, tag="sel25", name="sel25")
            nc.sync.dma_start(out=sel25, in_=sel25_d)

            # ---------- GroupNorm helpers ----------
            def affine_rg(g2, rows):
                """g2: [rows,2] SBUF (sum,sumsq) -> rg [rows,2] =
                (rstd, -mu*rstd)."""
                m2 = stat.tile([P, 2], f32, tag="m2", name="m2")[0:rows, :]
                nc.scalar.activation(out=m2, in_=g2, func=AF.Copy,
                                     scale=INV_CNT)
                rg = stat.tile([P, 2], f32, tag="rg", name="rg")[0:rows, :]
                nc.vector.tensor_tensor(out=rg[:, 0:1], in0=m2[:, 0:1],
                                        in1=m2[:, 0:1], op=MULT)
                nc.vector.tensor_tensor(out=rg[:, 0:1], in0=m2[:, 1:2],
                                        in1=rg[:, 0:1], op=SUB)
                nc.scalar.activation(out=rg[:, 0:1], in_=rg[:, 0:1],
                                     func=AF.Sqrt, bias=eps128[0:rows, :],
                                     scale=1.0)
                nc.vector.reciprocal(rg[:, 0:1], rg[:, 0:1])
                nc.vector.tensor_scalar(out=rg[:, 1:2], in0=m2[:, 0:1],
                                        scalar1=rg[:, 0:1], scalar2=-1.0,
                                        op0=MULT, op1=MULT)
                return rg

            def affine_apply(rg32, gamma, beta16, tag, lhsT=None):
                """rg32: [rows,2] at base partition 0 -> scale/shift [P, CB]."""
                ps_bc = psS.tile([P, 512], f32, tag="sm", name="psbc_" + tag)
                nc.tensor.matmul(ps_bc[:, 0:2], bc16 if lhsT is None else lhsT,
                                 rg32, start=True, stop=True)
                sc = stat.tile([P, CB], f32, tag="sc" + tag, name="sc" + tag)
                sh = stat.tile([P, CB], f32, tag="sh" + tag, name="sh" + tag)
                nc.vector.tensor_scalar_mul(out=sc, in0=gamma,
                                            scalar1=ps_bc[:, 0:1])
                nc.vector.scalar_tensor_tensor(out=sh, in0=gamma,
                                               scalar=ps_bc[:, 1:2],
                                               in1=beta16, op0=MULT, op1=ADD)
                return sc, sh

            def gn_stats(src, sq_dve, sums_name):
                """per-channel (sum, sumsq) of [P, CB, n] -> sums [P, CB, 2]"""
                sums = spat.tile([P, CB, 2], f32, tag="sums", name=sums_name,
                                 bufs=2)
                nc.vector.tensor_reduce(out=sums[:, :, 0:1], in_=src,
                                        axis=AX.X, op=ADD)
                n = src.shape[2]
                sq = spat.tile([P, HW], bf16, tag="sqj", name="sq_" + sums_name,
                               bufs=2)
                for j in range(CB):
                    if sq_dve:
                        nc.vector.tensor_tensor_reduce(
                            out=sq[:, 0:n], in0=src[:, j, :], in1=src[:, j, :],
                            scale=1.0, scalar=0.0, op0=MULT, op1=ADD,
                            accum_out=sums[:, j, 1:2])
                    else:
                        nc.scalar.activation(out=sq[:, 0:n], in_=src[:, j, :],
                                             func=AF.Square,
                                             accum_out=sums[:, j, 1:2])
                return sums

            def group_sums(sums, out_rows):
                """sums [P,CB,2] -> write [32,2] group sums into out_rows."""
                ps_g = psS.tile([P, 512], f32, tag="sm", name="psg")
                nc.tensor.matmul(ps_g[0:G, 0:2 * CB], sel4,
                                 sums.rearrange("p j s -> p (j s)"),
                                 start=True, stop=True)
                nc.vector.tensor_reduce(
                    out=out_rows,
                    in_=ps_g[0:G, 0:2 * CB].rearrange("g (j s) -> g s j", s=2),
                    axis=AX.X, op=ADD)

            xfs = [None] * T
            xfs[0] = xf0
            hns = [None] * T
            spatio_tiles = [None] * T
            gnt = [None] * T
            g2b3 = stat.tile([3 * G, 2], f32, tag="g2b3", name="g2b3", bufs=1)
            g2b2 = stat.tile([2 * G, 2], f32, tag="g2b2", name="g2b2", bufs=1)

            def gn_s(f):
                sums = gn_stats(xfs[f], False, f"sums_s{f}")
                g2s = stat.tile([G, 2], f32, tag="g2s", name=f"g2s{f}")
                group_sums(sums, g2s)
                rg = affine_rg(g2s, G)
                return affine_apply(rg, gam_sb["gamma_s"], gam_sb["beta_s16"],
                                    "s")

            def hn_make(f, sc, sh):
                hn = spat.tile([P, CB, HW], fp8, tag="hn", name=f"hn{f}",
                               bufs=2)
                for j in range(CB):
                    nc.scalar.activation(out=hn[:, j, :], in_=xfs[f][:, j, :],
                                         func=AF.Identity,
                                         scale=sc[:, j:j + 1],
                                         bias=sh[:, j:j + 1])
                hns[f] = hn

            def load_x(f):
                xf = spat.tile([P, CB, HW], f32, tag="xf", name=f"xf{f}",
                               bufs=2)
                nc.sync.dma_start(
                    out=xf, in_=x_d[f].rearrange("(p j) hw -> p j hw", p=P))
                xfs[f] = xf

            # ================= spatial phase =================
            sc_s, sh_s = gn_s(0)
            for nm in ["wo", "wqt", "wkt", "wvt"]:
                w_sb[nm] = consts.tile([P, CB, C], fp8, tag="w_" + nm,
                                       name="w_" + nm)
                nc.sync.dma_start(
                    out=w_sb[nm],
                    in_=w_d[nm].rearrange("(p kc) co -> p kc co", p=P))
            wot_sb = consts.tile([P, CB, C], fp8, tag="w_wot", name="w_wot")
            nc.sync.dma_start(
                out=wot_sb, in_=wot_d.rearrange("(p kc) co -> p kc co", p=P))
            hn_make(0, sc_s, sh_s)

            for f in range(T):
                hn = hns[f]
                if f + 1 < T:
                    load_x(f + 1)

                # ---- k conv: [P, CB, HW] fp8 = 4*k ----
                k_sb = spat.tile([P, CB, HW], fp8, tag="k_sb", name="k_sb")
                for jp in (0, 2):
                    for h in (0, 1):
                        ps = psA.tile([P, 2, 512], f32, tag="big", name="psk")
                        for i in (0, 1):
                            for kcp in (0, 1):
                                nc.tensor.matmul(
                                    ps[:, i, :],
                                    w_sb["wk"][:, 2 * kcp:2 * kcp + 2,
                                               (jp + i) * P:(jp + i + 1) * P],
                                    hn[:, 2 * kcp:2 * kcp + 2,
                                       h * 512:(h + 1) * 512],
                                    start=(kcp == 0), stop=(kcp == 1),
                                    perf_mode=DR)
                        nc.vector.tensor_scalar(
                            out=k_sb[:, jp:jp + 2, h * 512:(h + 1) * 512],
                            in0=ps, scalar1=1.0 / 64.0, scalar2=0.0,
                            op0=MULT, op1=ADD)

                # ---- vT conv: [P(kpix), KB, C] fp8 = 4*v ----
                vT = spat.tile([P, KB, C], fp8, tag="vT", name="vT")
                for pp in (0, 2, 4, 6):
                    ps = psA.tile([P, 2, 512], f32, tag="big", name="psv")
                    for i in (0, 1):
                        for kcp in (0, 1):
                            nc.tensor.matmul(
                                ps[:, i, :],
                                hn[:, 2 * kcp:2 * kcp + 2,
                                   (pp + i) * P:(pp + i + 1) * P],
                                w_sb["wv"][:, 2 * kcp:2 * kcp + 2, :],
                                start=(kcp == 0), stop=(kcp == 1),
                                perf_mode=DR)
                    nc.vector.tensor_scalar(out=vT[:, pp:pp + 2, :], in0=ps,
                                            scalar1=1.0 / 64.0, scalar2=0.0,
                                            op0=MULT, op1=ADD)

                # stats for next frame overlap the conv matmuls
                if f + 1 < T:
                    sc_s, sh_s = gn_s(f + 1)

                # ---- q conv: [P, CB, HALF] fp8 = 4*q ----
                q_sb = spat.tile([P, CB, HALF], fp8, tag="q_sb", name="q_sb")
                for jp in (0, 2):
                    ps = psA.tile([P, 2, 512], f32, tag="big", name="psq")
                    for i in (0, 1):
                        for kcp in (0, 1):
                            nc.tensor.matmul(
                                ps[:, i, :],
                                w_sb["wq"][:, 2 * kcp:2 * kcp + 2,
                                           (jp + i) * P:(jp + i + 1) * P],
                                hn[:, 2 * kcp:2 * kcp + 2, 0:HALF],
                                start=(kcp == 0), stop=(kcp == 1),
                                perf_mode=DR)
                    nc.scalar.activation(out=q_sb[:, jp:jp + 2, :], in_=ps,
                                         func=AF.Copy, scale=1.0 / 64.0)

                if f + 1 < T:
                    hn_make(f + 1, sc_s, sh_s)

                # ---- scoresT + exp: expT [P(kpix), KB, HALF] fp8 ----
                expT = spat.tile([P, KB, HALF], fp8, tag="expT", name="expT")
                for kp in (0, 2, 4, 6):
                    ps = psA.tile([P, 2, 512], f32, tag="big", name="pssc")
                    for i in (0, 1):
                        for kcp in (0, 1):
                            nc.tensor.matmul(
                                ps[:, i, :],
                                k_sb[:, 2 * kcp:2 * kcp + 2,
                                     (kp + i) * P:(kp + i + 1) * P],
                                q_sb[:, 2 * kcp:2 * kcp + 2, :],
                                start=(kcp == 0), stop=(kcp == 1),
                                perf_mode=DR)
                    nc.scalar.activation(out=expT[:, kp:kp + 2, :], in_=ps,
                                         func=AF.Exp, scale=SCALE / 16.0)

                # ---- den = sum_k exp ; rdenB = 2/den bcast ----
                ps_den = psS.tile([P, 512], f32, tag="sm", name="psden")
                for i in range(KB):
                    nc.tensor.matmul(ps_den[0:1, :], ones1f,
                                     expT[:, i, :],
                                     start=(i == 0), stop=(i == KB - 1))
                rden = spat.tile([1, 512], bf16, tag="rden", name="rden")
                nc.vector.reciprocal(rden, ps_den[0:1, :])
                rden_dr = dram.tile([512], bf16, tag="rden_dr",
                                    name="rden_dr")
                nc.sync.dma_start(out=rden_dr[:], in_=rden)
                rdenB = spat.tile([P, 512], bf16, tag="rdenB", name="rdenB")
                nc.sync.dma_start(
                    out=rdenB,
                    in_=rden_dr[:].unsqueeze(0).to_broadcast([P, 512]))

                # ---- hsp = vT'.T @ expT : [P, CB, HALF] fp8 = hsp_u/32 ----
                hsp = spat.tile([P, CB, HALF], fp8, tag="hsp", name="hsp")
                for cp in (0, 2):
                    ps = psA.tile([P, 2, 512], f32, tag="big", name="psh")
                    for i in (0, 1):
                        for kbp in range(4):
                            nc.tensor.matmul(
                                ps[:, i, :],
                                vT[:, 2 * kbp:2 * kbp + 2,
                                   (cp + i) * P:(cp + i + 1) * P],
                                expT[:, 2 * kbp:2 * kbp + 2, :],
                                start=(kbp == 0), stop=(kbp == 3),
                                perf_mode=DR)
                    nc.vector.tensor_scalar(out=hsp[:, cp:cp + 2, :], in0=ps,
                                            scalar1=1.0 / 128.0, scalar2=0.0,
                                            op0=MULT, op1=ADD)

                # ---- wo conv + deferred normalize + residual ----
                spatio = spatio_p.tile([P, CB, HALF], bf16, tag="spatio",
                                       name=f"spatio{f}")
                for cp in (0, 2):
                    ps = psA.tile([P, 2, 512], f32, tag="big", name="pso")
                    for i in (0, 1):
                        for kcp in (0, 1):
                            nc.tensor.matmul(
                                ps[:, i, :],
                                w_sb["wo"][:, 2 * kcp:2 * kcp + 2,
                                           (cp + i) * P:(cp + i + 1) * P],
                                hsp[:, 2 * kcp:2 * kcp + 2, :],
                                start=(kcp == 0), stop=(kcp == 1),
                                perf_mode=DR)
                    s_n = spat.tile([P, 2, 512], bf16, tag="s_n", name="s_n",
                                    bufs=2)
                    nc.vector.tensor_tensor(
                        out=s_n, in0=ps,
                        in1=rdenB.unsqueeze(1).to_broadcast([P, 2, 512]),
                        op=MULT)
                    nc.gpsimd.tensor_tensor(out=spatio[:, cp:cp + 2, :],
                                            in0=s_n,
                                            in1=xfs[f][:, cp:cp + 2, 0:512],
                                            op=ADD)

                spatio_tiles[f] = spatio

                # ---- GN_t partial stats ----
                sums_t = gn_stats(spatio, False, f"sums_t{f}")
                if f < 3:
                    group_sums(sums_t, g2b3[f * G:(f + 1) * G, :])
                else:
                    group_sums(sums_t, g2b2[(f - 3) * G:(f - 2) * G, :])

                if f == 2:
                    bnc_in3 = dram.tile([3 * G, 2], f32, tag="bnc_in3",
                                        name="bnc_in3")
                    bnc_out3 = dram.tile([3 * G, 2], f32, tag="bnc_out3",
                                         name="bnc_out3")
                    nc.sync.dma_start(out=bnc_in3[:], in_=g2b3[:])
                    nc.gpsimd.collective_compute(
                        "AllReduce", ADD,
                        replica_groups=[[0, 1], [2, 3], [4, 5], [6, 7]],
                        ins=[bnc_in3.opt()], outs=[bnc_out3.opt()])

            # frames 3-4 collective
            bnc_in2 = dram.tile([2 * G, 2], f32, tag="bnc_in2", name="bnc_in2")
            bnc_out2 = dram.tile([2 * G, 2], f32, tag="bnc_out2",
                                 name="bnc_out2")
            nc.sync.dma_start(out=bnc_in2[:], in_=g2b2[:])
            nc.gpsimd.collective_compute(
                "AllReduce", ADD,
                replica_groups=[[0, 1], [2, 3], [4, 5], [6, 7]],
                ins=[bnc_in2.opt()], outs=[bnc_out2.opt()])

            # ---- temporal GN affines (frames 0-2 batched) + gnt ----
            spat_cm.__exit__(None, None, None)
            tempo_cm = tc.tile_pool(name="tempo", bufs=1)
            tempo = tempo_cm.__enter__()
            gsum3 = stat.tile([3 * G, 2], f32, tag="gsum3", name="gsum3",
                              bufs=1)
            nc.sync.dma_start(out=gsum3[:], in_=bnc_out3[:])
            rg3 = affine_rg(gsum3, 3 * G)

            def gnt_make(f, rg32, lhsT=None):
                sc, sh = affine_apply(rg32, gam_sb["gamma_t"],
                                      gam_sb["beta_t16"], "t", lhsT=lhsT)
                g = gnt_p.tile([P, CB, HALF], fp8, tag="gnt", name=f"gnt{f}")
                for j in range(CB):
                    nc.vector.tensor_scalar(out=g[:, j, :],
                                            in0=spatio_tiles[f][:, j, :],
                                            scalar1=sc[:, j:j + 1],
                                            scalar2=sh[:, j:j + 1],
                                            op0=MULT, op1=ADD)
                gnt[f] = g

            # ================= temporal phase =================
            q5c = tempo.tile([P, T, CB, 512], bf16, tag="q5c", name="q5c")
            k5c = tempo.tile([P, T, CB, 512], bf16, tag="k5c", name="k5c")
            v5c = tempo.tile([P, T, CB, 512], bf16, tag="v5c", name="v5c")

            def tconv(t):
                for nm, dst in (("wqt", q5c), ("wkt", k5c), ("wvt", v5c)):
                    for jp in (0, 2):
                        ps = psA.tile([P, 2, 512], f32, tag="big", name="pst")
                        for i in (0, 1):
                            for kcp in (0, 1):
                                nc.tensor.matmul(
                                    ps[:, i, :],
                                    w_sb[nm][:, 2 * kcp:2 * kcp + 2,
                                             (jp + i) * P:(jp + i + 1) * P],
                                    gnt[t][:, 2 * kcp:2 * kcp + 2, :],
                                    start=(kcp == 0), stop=(kcp == 1),
                                    perf_mode=DR)
                        nc.scalar.activation(out=dst[:, t, jp:jp + 2, :],
                                             in_=ps, func=AF.Copy,
                                             scale=1.0 / 64.0)

            ps_sc = psS.tile([P, 512], f32, tag="sm", name="ps_sc")
            pair_seq = sorted([(t, s) for t in range(T) for s in range(T)],
                              key=lambda p: max(p))
            pair_idx = [0]

            def em_pairs(upto):
                while pair_idx[0] < len(pair_seq):
                    t, s = pair_seq[pair_idx[0]]
                    if max(t, s) > upto:
                        return
                    em = tempo.tile([P, CB, 512], bf16, tag="em", name="em",
                                    bufs=2)
                    nc.vector.tensor_tensor(out=em, in0=q5c[:, t, :, :],
                                            in1=k5c[:, s, :, :], op=MULT)
                    ts_ = 5 * t + s
                    first = pair_idx[0] == 0
                    last = pair_idx[0] == len(pair_seq) - 1
                    for kc in range(CB):
                        nc.tensor.matmul(
                            ps_sc[0:25, :], sel25[:, ts_, :],
                            em[:, kc, :],
                            start=(first and kc == 0),
                            stop=(last and kc == 3))
                    pair_idx[0] += 1

            for f in range(3):
                gnt_make(f, rg3, lhsT=bc16f[0:3 * G, f, :])
            for t in range(3):
                tconv(t)
                em_pairs(t)
            gsum2 = stat.tile([2 * G, 2], f32, tag="gsum2", name="gsum2",
                              bufs=1)
            nc.sync.dma_start(out=gsum2[:], in_=bnc_out2[:])
            rg2 = affine_rg(gsum2, 2 * G)
            for f in (3, 4):
                gnt_make(f, rg2, lhsT=bc16f[0:2 * G, f, :])
                tconv(f)
                em_pairs(f)
            exp_sb = tempo.tile([32, 512], bf16, tag="exp_sb", name="exp_sb")
            nc.vector.memset(exp_sb, 0.0)
            nc.scalar.activation(out=exp_sb[0:25, :], in_=ps_sc[0:25, :],
                                 func=AF.Exp, scale=SCALE / 16.0)
            ps_d5 = psS.tile([P, 512], f32, tag="sm", name="ps_d5")
            nc.tensor.matmul(ps_d5[0:5, :], ind5, exp_sb, start=True,
                             stop=True)
            rden5 = tempo.tile([16, 512], bf16, tag="rden5", name="rden5")
            nc.vector.memset(rden5, 0.0)
            nc.vector.reciprocal(rden5[0:5, :], ps_d5[0:5, :])
            a5P = tempo.tile([P, QB, 32], bf16, tag="a5P", name="a5P")
            nc.sync.dma_start(out=a5P, in_=exp_sb, transpose=True)
            rdenP = tempo.tile([P, QB, 16], bf16, tag="rdenP", name="rdenP")
            nc.sync.dma_start(out=rdenP, in_=rden5, transpose=True)
            a5n = tempo.tile([P, QB, 32], bf16, tag="a5n", name="a5n")
            nc.vector.memset(a5n, 0.0)
            nc.vector.tensor_tensor(
                out=a5n[:, :, 0:25].rearrange("p q (t s) -> p q t s", t=5),
                in0=a5P[:, :, 0:25].rearrange("p q (t s) -> p q t s", t=5),
                in1=rdenP[:, :, 0:5].unsqueeze(3).to_broadcast([P, QB, 5, 5]),
                op=MULT)
            a_n2 = tempo.tile([P, P], bf16, tag="a_n2", name="a_n2")
            nc.sync.dma_start(out=a_n2,
                              in_=a5n.rearrange("p q w -> p (q w)"),
                              transpose=True)
            a_dr = dram.tile([32, QB, P], bf16, tag="a_dr", name="a_dr")
            nc.sync.dma_start(
                out=a_dr[:].rearrange("w q c -> q w c"),
                in_=a_n2)

            # ---- htp + wot + residual out, frame by frame ----
            xh0 = tempo.tile([P, CB, HALF], f32, tag="xh", name="xh0", bufs=2)
            nc.sync.dma_start(
                out=xh0, in_=x_d[0][:, 0:HALF].rearrange("(p j) hw -> p j hw",
                                                         p=P))
            xhs = [xh0, None, None, None, None]

            def bc_load(t):
                row = []
                for s in range(T):
                    ab = tempo.tile([P, 512], bf16, tag="a_bc", name="a_bc",
                                    bufs=8)
                    nc.sync.dma_start(
                        out=ab.rearrange("p (q c) -> p q c", q=QB),
                        in_=a_dr[5 * t + s, :, :].unsqueeze(0)
                        .to_broadcast([P, QB, P]))
                    row.append(ab)
                return row

            abc = [None] * T
            abc[0] = bc_load(0)
            for t in range(T):
                if t + 1 < T:
                    xh = tempo.tile([P, CB, HALF], f32, tag="xh",
                                    name=f"xh{t + 1}", bufs=2)
                    nc.sync.dma_start(
                        out=xh,
                        in_=x_d[t + 1][:, 0:HALF].rearrange(
                            "(p j) hw -> p j hw", p=P))
                    xhs[t + 1] = xh
                    abc[t + 1] = bc_load(t + 1)
                u = []
                for s in range(T):
                    us = tempo.tile([P, CB, 512], bf16, tag="u", name="u",
                                    bufs=5)
                    nc.vector.tensor_tensor(
                        out=us, in0=v5c[:, s, :, :],
                        in1=abc[t][s].unsqueeze(1).to_broadcast([P, CB, 512]),
                        op=MULT)
                    u.append(us)
                w01 = tempo.tile([P, CB, 512], bf16, tag="w01", name="w01",
                                 bufs=2)
                nc.vector.tensor_tensor(out=w01, in0=u[0], in1=u[1], op=ADD)
                w23 = tempo.tile([P, CB, 512], bf16, tag="w23", name="w23",
                                 bufs=2)
                nc.vector.tensor_tensor(out=w23, in0=u[2], in1=u[3], op=ADD)
                nc.vector.tensor_tensor(out=w01, in0=w01, in1=w23, op=ADD)
                htpT = tempo.tile([P, CB, HALF], bf16, tag="htpT",
                                  name="htpT", bufs=2)
                nc.vector.tensor_tensor(out=htpT, in0=w01, in1=u[4], op=ADD)
                htp8 = tempo.tile([P, CB, HALF], fp8, tag="htp8",
                                  name="htp8", bufs=2)
                nc.scalar.activation(out=htp8, in_=htpT, func=AF.Copy,
                                     scale=4.0)
                for cp in (0, 2):
                    ps = psA.tile([P, 2, 512], f32, tag="big", name="psw")
                    for i in (0, 1):
                        for kcp in (0, 1):
                            nc.tensor.matmul(
                                ps[:, i, :],
                                wot_sb[:, 2 * kcp:2 * kcp + 2,
                                       (cp + i) * P:(cp + i + 1) * P],
                                htp8[:, 2 * kcp:2 * kcp + 2, :],
                                start=(kcp == 0), stop=(kcp == 1),
                                perf_mode=DR)
                    o1 = tempo.tile([P, 2, 512], bf16, tag="o1", name="o1",
                                    bufs=2)
                    nc.scalar.activation(out=o1, in_=ps, func=AF.Copy,
                                         scale=1.0 / 256.0)
                    oc = tempo.tile([P, 2, 512], f32, tag="oc", name="oc",
                                    bufs=2)
                    nc.gpsimd.tensor_tensor(out=oc, in0=o1,
                                            in1=xhs[t][:, cp:cp + 2, :],
                                            op=ADD)
                    nc.sync.dma_start(
                        out=out_d[t].rearrange("(p j) hw -> p j hw",
                                               p=P)[:, cp:cp + 2, :],
                        in_=oc)
            tempo_cm.__exit__(None, None, None)

    nc.compile()
    return nc


# storage column s holds natural channel 4*(s % 128) + s // 128
_COL_PERM = np.array([4 * (s % P) + s // P for s in range(C)])


def _prepare_in_maps(inputs):
    import ml_dtypes
    fp8 = ml_dtypes.float8_e4m3
    x = np.asarray(inputs["x"], np.float32).reshape(B * T, C, HW)
    sel4 = np.zeros((P, G), np.float32)
    for p in range(P):
        sel4[p, p // 4] = 1.0
    bcast16 = sel4.T.copy() * 16.0
    common = {}
    for nm in ["wq", "wk", "wv", "wo", "wqt", "wkt", "wvt"]:
        w = np.asarray(inputs[nm], np.float32)   # [out, in]
        wt = np.ascontiguousarray(w.T[:, _COL_PERM]) * 16.0
        common[nm + "T"] = np.clip(wt, -240.0, 240.0).astype(fp8)
    wot = np.asarray(inputs["wot"], np.float32)
    common["wotT"] = np.clip(
        np.ascontiguousarray(wot.T[:, _COL_PERM]) * 16.0,
        -240.0, 240.0).astype(fp8)
    common["gamma_s"] = np.asarray(inputs["gamma_s"], np.float32)
    common["beta_s16"] = 16.0 * np.asarray(inputs["beta_s"], np.float32)
    common["gamma_t"] = np.asarray(inputs["gamma_t"], np.float32)
    common["beta_t16"] = 16.0 * np.asarray(inputs["beta_t"], np.float32)
    common["sel4"] = sel4
    common["bcast16"] = bcast16
    bc16f = np.zeros((P, 5, P), np.float32)
    base = [0, G, 2 * G, 0, G]
    for f in range(5):
        for p in range(P):
            bc16f[base[f] + p // 4, f, p] = 16.0
    common["bcast16f"] = bc16f
    ind5 = np.zeros((32, 5), np.float32)
    for t in range(5):
        for s in range(5):
            ind5[5 * t + s, t] = 1.0
    common["ind5"] = ind5.astype(ml_dtypes.bfloat16)
    sel25 = np.zeros((P, 25, 25), np.float32)
    for ts_ in range(25):
        sel25[:, ts_, ts_] = 1.0
    common["sel25"] = sel25.astype(ml_dtypes.bfloat16)

    in_maps = []
    for v in range(B):
        xv = x[v * T:(v + 1) * T]
        for h in range(2):
            if h == 0:
                xc = xv
            else:
                xc = np.concatenate([xv[..., HALF:], xv[..., :HALF]], axis=-1)
            m = dict(common)
            m["x"] = np.ascontiguousarray(xc)
            in_maps.append(m)
    return in_maps


def _run(inputs, trace=False):
    from concourse import bass_utils
    if "nc" not in _CACHE:
        _CACHE["nc"] = _build()
    nc = _CACHE["nc"]
    in_maps = _prepare_in_maps(inputs)
    if trace:
        try:
            from antenv.axon_hooks import get_axon_ntff_profile_hook  # noqa: F401
        except ModuleNotFoundError:
            trace = False
    res = bass_utils.run_bass_kernel_spmd(nc, in_maps, core_ids=list(range(8)),
                                          trace=trace)
    out = np.empty((B * T, C, HW), np.float32)
    for v in range(B):
        for h in range(2):
            o = res.results[2 * v + h]["out"]  # [T, C, HALF]
            if h == 0:
                out[v * T:(v + 1) * T, :, :HALF] = o
            else:
                out[v * T:(v + 1) * T, :, HALF:] = o
    return out.reshape(B * T, C, 32, 32), res


def kernel(**inputs) -> np.ndarray:
    out, _ = _run(inputs, trace=False)
    return out


# revision 48
# speedup vs baseline: 1.0269x; 1.0269x over previous
"""Trainium2 Bass kernel for nn_AttnBlock_Spatio_Temporal (B=4,T=5,C=512,H=W=32).

Distribution: 8 cores = (video b in 0..3) x (pixel-half h in 0..1); host rolls
the HW axis per core so its own 512 pixels come first. All heavy matmuls run
in fp8e4 DoubleRow (K=256 per MM). Spatial attention computes scores
TRANSPOSED (keys on partitions) so no attention transpose is needed; softmax
normalization is deferred through the v- and wo-matmuls and applied once per
output pixel (PE broadcast of 1/den). GroupNorm_t cross-half stats use two
batched pair-AllReduces (frames 0-3 in one [128,2] op, frame 4 in [32,2]).

Channel-major layout: channel c lives at (partition p, block j), c = 4p + j.
Weight matrices are host-permuted (columns) and pre-scaled x16 into fp8.
"""
import numpy as np

B, T, C, HW = 4, 5, 512, 1024
G = 32
EPS = 1e-6
P = 128
CB = C // P          # 4 channel blocks
HALF = HW // 2       # 512 own pixels
KB = HW // P         # 8 key-pixel blocks
QB = HALF // P       # 4 query/pixel blocks
SCALE = float(C) ** -0.5
INV_CNT = 1.0 / 16384.0   # per-group element count (16ch * 1024px)

_CACHE = {}


def _build():
    import concourse.bacc as bacc
    import concourse.tile as tile
    import concourse.mybir as mybir

    f32 = mybir.dt.float32
    bf16 = mybir.dt.bfloat16
    fp8 = mybir.dt.float8e4
    MULT = mybir.AluOpType.mult
    ADD = mybir.AluOpType.add
    SUB = mybir.AluOpType.subtract
    AF = mybir.ActivationFunctionType
    AX = mybir.AxisListType
    DR = mybir.MatmulPerfMode.DoubleRow

    nc = bacc.Bacc("TRN2", target_bir_lowering=False, debug=False, num_devices=8)

    x_d = nc.dram_tensor("x", [T, C, HW], f32, kind="ExternalInput").ap()
    fp8_w = ["wq", "wk", "wv", "wo", "wqt", "wkt", "wvt"]
    w_d = {nm: nc.dram_tensor(nm + "T", [C, C], fp8, kind="ExternalInput").ap()
           for nm in fp8_w}
    wot_d = nc.dram_tensor("wotT", [C, C], fp8, kind="ExternalInput").ap()
    g_d = {nm: nc.dram_tensor(nm, [C], f32, kind="ExternalInput").ap()
           for nm in ["gamma_s", "beta_s16", "gamma_t", "beta_t16"]}
    sel_d = nc.dram_tensor("sel4", [P, G], f32, kind="ExternalInput").ap()
    bc16_d = nc.dram_tensor("bcast16", [G, P], f32, kind="ExternalInput").ap()
    bc16f_d = nc.dram_tensor("bcast16f", [P, 5, P], f32,
                             kind="ExternalInput").ap()
    ind5_d = nc.dram_tensor("ind5", [32, 5], bf16, kind="ExternalInput").ap()
    sel25_d = nc.dram_tensor("sel25", [P, 25, 25], bf16,
                             kind="ExternalInput").ap()
    out_d = nc.dram_tensor("out", [T, C, HALF], f32, kind="ExternalOutput").ap()

    def cpart(ap_1d):  # [C] dram -> [128, CB] tile order (c = 4p + j)
        return ap_1d.rearrange("(p j) -> p j", p=P)

    with tile.TileContext(nc) as tc, \
         nc.allow_low_precision("fp8/bf16 pipeline by design"):
        with tc.tile_pool(name="consts", bufs=1) as consts, \
             tc.tile_pool(name="stat", bufs=2) as stat, \
             tc.tile_pool(name="spatio_p", bufs=T) as spatio_p, \
             tc.tile_pool(name="gnt_p", bufs=T) as gnt_p, \
             tc.tile_pool(name="psA", bufs=3, space="PSUM") as psA, \
             tc.tile_pool(name="psS", bufs=2, space="PSUM") as psS, \
             tc.tile_pool(name="dram", bufs=4, space="DRAM") as dram:

            # ---------------- constants ----------------
            spat_cm = tc.tile_pool(name="spat", bufs=1)
            spat = spat_cm.__enter__()
            nc.sync.dma_start(
                out=(xf0 := spat.tile([P, CB, HW], f32, tag="xf", name="xf0",
                                      bufs=2)),
                in_=x_d[0].rearrange("(p j) hw -> p j hw", p=P))
            w_sb = {}
            for nm in ["wk", "wq", "wv"]:
                w_sb[nm] = consts.tile([P, CB, C], fp8, tag="w_" + nm,
                                       name="w_" + nm)
                nc.sync.dma_start(
                    out=w_sb[nm],
                    in_=w_d[nm].rearrange("(p kc) co -> p kc co", p=P))
            gam_sb = {}
            for nm in ["gamma_s", "beta_s16", "gamma_t", "beta_t16"]:
                gam_sb[nm] = consts.tile([P, CB], f32, tag="g_" + nm,
                                         name="g_" + nm)
                nc.sync.dma_start(out=gam_sb[nm], in_=cpart(g_d[nm]))
            sel4 = consts.tile([P, G], f32, tag="sel4", name="sel4")
            nc.sync.dma_start(out=sel4, in_=sel_d)
            bc16 = consts.tile([G, P], f32, tag="bc16", name="bc16")
            nc.sync.dma_start(out=bc16, in_=bc16_d)
            bc16f = consts.tile([P, 5, P], f32, tag="bc16f", name="bc16f")
            nc.sync.dma_start(out=bc16f, in_=bc16f_d)
            eps128 = consts.tile([P, 1], f32, tag="eps", name="eps128")
            nc.vector.memset(eps128, EPS)
            ones1f = consts.tile([P, 1], fp8, tag="ones1f", name="ones1f")
            nc.vector.memset(ones1f, 1.0)
            ind5 = consts.tile([32, 5], bf16, tag="ind5", name="ind5")
            nc.sync.dma_start(out=ind5, in_=ind5_d)
            sel25 = consts.tile([P, 25, 25], bf16, tag="sel25", name="sel25")
            nc.sync.dma_start(out=sel25, in_=sel25_d)

            # ---------- GroupNorm helpers ----------
            def affine_rg(g2, rows):
                """g2: [rows,2] SBUF (sum,sumsq) -> rg [rows,2] =
                (rstd, -mu*rstd)."""
                m2 = stat.tile([P, 2], f32, tag="m2", name="m2")[0:rows, :]
                nc.scalar.activation(out=m2, in_=g2, func=AF.Copy,
                                     scale=INV_CNT)
                rg = stat.tile([P, 2], f32, tag="rg", name="rg")[0:rows, :]
                nc.vector.tensor_tensor(out=rg[:, 0:1], in0=m2[:, 0:1],
                                        in1=m2[:, 0:1], op=MULT)
                nc.vector.tensor_tensor(out=rg[:, 0:1], in0=m2[:, 1:2],
                                        in1=rg[:, 0:1], op=SUB)
                nc.scalar.activation(out=rg[:, 0:1], in_=rg[:, 0:1],
                                     func=AF.Sqrt, bias=eps128[0:rows, :],
                                     scale=1.0)
                nc.vector.reciprocal(rg[:, 0:1], rg[:, 0:1])
                nc.vector.tensor_scalar(out=rg[:, 1:2], in0=m2[:, 0:1],
                                        scalar1=rg[:, 0:1], scalar2=-1.0,
                                        op0=MULT, op1=MULT)
                return rg

            def affine_apply(rg32, gamma, beta16, tag, lhsT=None):
                """rg32: [rows,2] at base partition 0 -> scale/shift [P, CB]."""
                ps_bc = psS.tile([P, 512], f32, tag="sm", name="psbc_" + tag)
                nc.tensor.matmul(ps_bc[:, 0:2], bc16 if lhsT is None else lhsT,
                                 rg32, start=True, stop=True)
                sc = stat.tile([P, CB], f32, tag="sc" + tag, name="sc" + tag)
                sh = stat.tile([P, CB], f32, tag="sh" + tag, name="sh" + tag)
                nc.vector.tensor_scalar_mul(out=sc, in0=gamma,
                                            scalar1=ps_bc[:, 0:1])
                nc.vector.scalar_tensor_tensor(out=sh, in0=gamma,
                                               scalar=ps_bc[:, 1:2],
                                               in1=beta16, op0=MULT, op1=ADD)
                return sc, sh

            def gn_stats(src, sq_dve, sums_name):
                """per-channel (sum, sumsq) of [P, CB, n] -> sums [P, CB, 2]"""
                sums = spat.tile([P, CB, 2], f32, tag="sums", name=sums_name,
                                 bufs=2)
                nc.vector.tensor_reduce(out=sums[:, :, 0:1], in_=src,
                                        axis=AX.X, op=ADD)
                n = src.shape[2]
                sq = spat.tile([P, HW], bf16, tag="sqj", name="sq_" + sums_name,
                               bufs=2)
                for j in range(CB):
                    if sq_dve:
                        nc.vector.tensor_tensor_reduce(
                            out=sq[:, 0:n], in0=src[:, j, :], in1=src[:, j, :],
                            scale=1.0, scalar=0.0, op0=MULT, op1=ADD,
                            accum_out=sums[:, j, 1:2])
                    else:
                        nc.scalar.activation(out=sq[:, 0:n], in_=src[:, j, :],
                                             func=AF.Square,
                                             accum_out=sums[:, j, 1:2])
                return sums

            def group_sums(sums, out_rows):
                """sums [P,CB,2] -> write [32,2] group sums into out_rows."""
                ps_g = psS.tile([P, 512], f32, tag="sm", name="psg")
                nc.tensor.matmul(ps_g[0:G, 0:2 * CB], sel4,
                                 sums.rearrange("p j s -> p (j s)"),
                                 start=True, stop=True)
                nc.vector.tensor_reduce(
                    out=out_rows,
                    in_=ps_g[0:G, 0:2 * CB].rearrange("g (j s) -> g s j", s=2),
                    axis=AX.X, op=ADD)

            xfs = [None] * T
            xfs[0] = xf0
            hns = [None] * T
            spatio_tiles = [None] * T
            gnt = [None] * T
            g2b3 = stat.tile([3 * G, 2], f32, tag="g2b3", name="g2b3", bufs=1)
            g2b2 = stat.tile([2 * G, 2], f32, tag="g2b2", name="g2b2", bufs=1)

            def gn_s(f):
                sums = gn_stats(xfs[f], False, f"sums_s{f}")
                g2s = stat.tile([G, 2], f32, tag="g2s", name=f"g2s{f}")
                group_sums(sums, g2s)
                rg = affine_rg(g2s, G)
                return affine_apply(rg, gam_sb["gamma_s"], gam_sb["beta_s16"],
                                    "s")

            def hn_make(f, sc, sh):
                hn = spat.tile([P, CB, HW], fp8, tag="hn", name=f"hn{f}",
                               bufs=2)
                for j in range(CB):
                    nc.scalar.activation(out=hn[:, j, :], in_=xfs[f][:, j, :],
                                         func=AF.Identity,
                                         scale=sc[:, j:j + 1],
                                         bias=sh[:, j:j + 1])
                hns[f] = hn

            def load_x(f):
                xf = spat.tile([P, CB, HW], f32, tag="xf", name=f"xf{f}",
                               bufs=2)
                nc.sync.dma_start(
                    out=xf, in_=x_d[f].rearrange("(p j) hw -> p j hw", p=P))
                xfs[f] = xf

            # ================= spatial phase =================
            sc_s, sh_s = gn_s(0)
            for nm in ["wo", "wqt", "wkt", "wvt"]:
                w_sb[nm] = consts.tile([P, CB, C], fp8, tag="w_" + nm,
                                       name="w_" + nm)
                nc.sync.dma_start(
                    out=w_sb[nm],
                    in_=w_d[nm].rearrange("(p kc) co -> p kc co", p=P))
            wot_sb = consts.tile([P, CB, C], fp8, tag="w_wot", name="w_wot")
            nc.sync.dma_start(
                out=wot_sb, in_=wot_d.rearrange("(p kc) co -> p kc co", p=P))
            hn_make(0, sc_s, sh_s)

            for f in range(T):
                hn = hns[f]
                if f + 1 < T:
                    load_x(f + 1)

                # ---- k conv: [P, CB, HW] fp8 = 4*k ----
                k_sb = spat.tile([P, CB, HW], fp8, tag="k_sb", name="k_sb")
                for jp in (0, 2):
                    for h in (0, 1):
                        ps = psA.tile([P, 2, 512], f32, tag="big", name="psk")
                        for i in (0, 1):
                            for kcp in (0, 1):
                                nc.tensor.matmul(
                                    ps[:, i, :],
                                    w_sb["wk"][:, 2 * kcp:2 * kcp + 2,
                                               (jp + i) * P:(jp + i + 1) * P],
                                    hn[:, 2 * kcp:2 * kcp + 2,
                                       h * 512:(h + 1) * 512],
                                    start=(kcp == 0), stop=(kcp == 1),
                                    perf_mode=DR)
                        nc.scalar.activation(
                            out=k_sb[:, jp:jp + 2, h * 512:(h + 1) * 512],
                            in_=ps, func=AF.Copy, scale=1.0 / 64.0)

                # ---- vT conv: [P(kpix), KB, C] fp8 = 4*v ----
                vT = spat.tile([P, KB, C], fp8, tag="vT", name="vT")
                for pp in (0, 2, 4, 6):
                    ps = psA.tile([P, 2, 512], f32, tag="big", name="psv")
                    for i in (0, 1):
                        for kcp in (0, 1):
                            nc.tensor.matmul(
                                ps[:, i, :],
                                hn[:, 2 * kcp:2 * kcp + 2,
                                   (pp + i) * P:(pp + i + 1) * P],
                                w_sb["wv"][:, 2 * kcp:2 * kcp + 2, :],
                                start=(kcp == 0), stop=(kcp == 1),
                                perf_mode=DR)
                    nc.vector.tensor_scalar(out=vT[:, pp:pp + 2, :], in0=ps,
                                            scalar1=1.0 / 64.0, scalar2=0.0,
                                            op0=MULT, op1=ADD)

                # stats for next frame overlap the conv matmuls
                if f + 1 < T:
                    sc_s, sh_s = gn_s(f + 1)

                # ---- q conv: [P, CB, HALF] fp8 = 4*q ----
                q_sb = spat.tile([P, CB, HALF], fp8, tag="q_sb", name="q_sb")
                for jp in (0, 2):
                    ps = psA.tile([P, 2, 512], f32, tag="big", name="psq")
                    for i in (0, 1):
                        for kcp in (0, 1):
                            nc.tensor.matmul(
                                ps[:, i, :],
                                w_sb["wq"][:, 2 * kcp:2 * kcp + 2,
                                           (jp + i) * P:(jp + i + 1) * P],
                                hn[:, 2 * kcp:2 * kcp + 2, 0:HALF],
                                start=(kcp == 0), stop=(kcp == 1),
                                perf_mode=DR)
                    nc.vector.tensor_scalar(out=q_sb[:, jp:jp + 2, :],
                                            in0=ps, scalar1=1.0 / 64.0,
                                            scalar2=0.0, op0=MULT, op1=ADD)

                if f + 1 < T:
                    hn_make(f + 1, sc_s, sh_s)

                # ---- scoresT + exp: expT [P(kpix), KB, HALF] fp8 ----
                expT = spat.tile([P, KB, HALF], fp8, tag="expT", name="expT")
                for kp in (0, 2, 4, 6):
                    ps = psA.tile([P, 2, 512], f32, tag="big", name="pssc")
                    for i in (0, 1):
                        for kcp in (0, 1):
                            nc.tensor.matmul(
                                ps[:, i, :],
                                k_sb[:, 2 * kcp:2 * kcp + 2,
                                     (kp + i) * P:(kp + i + 1) * P],
                                q_sb[:, 2 * kcp:2 * kcp + 2, :],
                                start=(kcp == 0), stop=(kcp == 1),
                                perf_mode=DR)
                    nc.scalar.activation(out=expT[:, kp:kp + 2, :], in_=ps,
                                         func=AF.Exp, scale=SCALE / 16.0)

                # ---- den = sum_k exp ; rdenB = 2/den bcast ----
                ps_den = psS.tile([P, 512], f32, tag="sm", name="psden")
                for i in range(KB):
                    nc.tensor.matmul(ps_den[0:1, :], ones1f,
                                     expT[:, i, :],
                                     start=(i == 0), stop=(i == KB - 1))
                rden = spat.tile([1, 512], bf16, tag="rden", name="rden")
                nc.vector.reciprocal(rden, ps_den[0:1, :])
                rden_dr = dram.tile([512], bf16, tag="rden_dr",
                                    name="rden_dr")
                nc.sync.dma_start(out=rden_dr[:], in_=rden)
                rdenB = spat.tile([P, 512], bf16, tag="rdenB", name="rdenB")
                nc.sync.dma_start(
                    out=rdenB,
                    in_=rden_dr[:].unsqueeze(0).to_broadcast([P, 512]))

                # ---- hsp = vT'.T @ expT : [P, CB, HALF] fp8 = hsp_u/32 ----
                hsp = spat.tile([P, CB, HALF], fp8, tag="hsp", name="hsp")
                for cp in (0, 2):
                    ps = psA.tile([P, 2, 512], f32, tag="big", name="psh")
                    for i in (0, 1):
                        for kbp in range(4):
                            nc.tensor.matmul(
                                ps[:, i, :],
                                vT[:, 2 * kbp:2 * kbp + 2,
                                   (cp + i) * P:(cp + i + 1) * P],
                                expT[:, 2 * kbp:2 * kbp + 2, :],
                                start=(kbp == 0), stop=(kbp == 3),
                                perf_mode=DR)
                    nc.vector.tensor_scalar(out=hsp[:, cp:cp + 2, :], in0=ps,
                                            scalar1=1.0 / 128.0, scalar2=0.0,
                                            op0=MULT, op1=ADD)

                # ---- wo conv + deferred normalize + residual ----
                spatio = spatio_p.tile([P, CB, HALF], bf16, tag="spatio",
                                       name=f"spatio{f}")
                for cp in (0, 2):
                    ps = psA.tile([P, 2, 512], f32, tag="big", name="pso")
                    for i in (0, 1):
                        for kcp in (0, 1):
                            nc.tensor.matmul(
                                ps[:, i, :],
                                w_sb["wo"][:, 2 * kcp:2 * kcp + 2,
                                           (cp + i) * P:(cp + i + 1) * P],
                                hsp[:, 2 * kcp:2 * kcp + 2, :],
                                start=(kcp == 0), stop=(kcp == 1),
                                perf_mode=DR)
                    s_n = spat.tile([P, 2, 512], bf16, tag="s_n", name="s_n",
                                    bufs=2)
                    nc.vector.tensor_tensor(
                        out=s_n, in0=ps,
                        in1=rdenB.unsqueeze(1).to_broadcast([P, 2, 512]),
                        op=MULT)
                    nc.gpsimd.tensor_tensor(out=spatio[:, cp:cp + 2, :],
                                            in0=s_n,
                                            in1=xfs[f][:, cp:cp + 2, 0:512],
                                            op=ADD)

                spatio_tiles[f] = spatio

                # ---- GN_t partial stats ----
                sums_t = gn_stats(spatio, False, f"sums_t{f}")
                if f < 3:
                    group_sums(sums_t, g2b3[f * G:(f + 1) * G, :])
                else:
                    group_sums(sums_t, g2b2[(f - 3) * G:(f - 2) * G, :])

                if f == 2:
                    bnc_in3 = dram.tile([3 * G, 2], f32, tag="bnc_in3",
                                        name="bnc_in3")
                    bnc_out3 = dram.tile([3 * G, 2], f32, tag="bnc_out3",
                                         name="bnc_out3")
                    nc.sync.dma_start(out=bnc_in3[:], in_=g2b3[:])
                    nc.gpsimd.collective_compute(
                        "AllReduce", ADD,
                        replica_groups=[[0, 1], [2, 3], [4, 5], [6, 7]],
                        ins=[bnc_in3.opt()], outs=[bnc_out3.opt()])

            # frames 3-4 collective
            bnc_in2 = dram.tile([2 * G, 2], f32, tag="bnc_in2", name="bnc_in2")
            bnc_out2 = dram.tile([2 * G, 2], f32, tag="bnc_out2",
                                 name="bnc_out2")
            nc.sync.dma_start(out=bnc_in2[:], in_=g2b2[:])
            nc.gpsimd.collective_compute(
                "AllReduce", ADD,
                replica_groups=[[0, 1], [2, 3], [4, 5], [6, 7]],
                ins=[bnc_in2.opt()], outs=[bnc_out2.opt()])

            # ---- temporal GN affines (frames 0-2 batched) + gnt ----
            spat_cm.__exit__(None, None, None)
            tempo_cm = tc.tile_pool(name="tempo", bufs=1)
            tempo = tempo_cm.__enter__()
            gsum3 = stat.tile([3 * G, 2], f32, tag="gsum3", name="gsum3",
                              bufs=1)
            nc.sync.dma_start(out=gsum3[:], in_=bnc_out3[:])
            rg3 = affine_rg(gsum3, 3 * G)

            def gnt_make(f, rg32, lhsT=None):
                sc, sh = affine_apply(rg32, gam_sb["gamma_t"],
                                      gam_sb["beta_t16"], "t", lhsT=lhsT)
                g = gnt_p.tile([P, CB, HALF], fp8, tag="gnt", name=f"gnt{f}")
                for j in range(CB):
                    nc.vector.tensor_scalar(out=g[:, j, :],
                                            in0=spatio_tiles[f][:, j, :],
                                            scalar1=sc[:, j:j + 1],
                                            scalar2=sh[:, j:j + 1],
                                            op0=MULT, op1=ADD)
                gnt[f] = g

            # ================= temporal phase =================
            q5c = tempo.tile([P, T, CB, 512], bf16, tag="q5c", name="q5c")
            k5c = tempo.tile([P, T, CB, 512], bf16, tag="k5c", name="k5c")
            v5c = tempo.tile([P, T, CB, 512], bf16, tag="v5c", name="v5c")

            def tconv(t):
                for nm, dst in (("wqt", q5c), ("wkt", k5c), ("wvt", v5c)):
                    for jp in (0, 2):
                        ps = psA.tile([P, 2, 512], f32, tag="big", name="pst")
                        for i in (0, 1):
                            for kcp in (0, 1):
                                nc.tensor.matmul(
                                    ps[:, i, :],
                                    w_sb[nm][:, 2 * kcp:2 * kcp + 2,
                                             (jp + i) * P:(jp + i + 1) * P],
                                    gnt[t][:, 2 * kcp:2 * kcp + 2, :],
                                    start=(kcp == 0), stop=(kcp == 1),
                                    perf_mode=DR)
                        nc.scalar.activation(out=dst[:, t, jp:jp + 2, :],
                                             in_=ps, func=AF.Copy,
                                             scale=1.0 / 64.0)

            ps_sc = psS.tile([P, 512], f32, tag="sm", name="ps_sc")
            pair_seq = sorted([(t, s) for t in range(T) for s in range(T)],
                              key=lambda p: max(p))
            pair_idx = [0]

            def em_pairs(upto):
                while pair_idx[0] < len(pair_seq):
                    t, s = pair_seq[pair_idx[0]]
                    if max(t, s) > upto:
                        return
                    em = tempo.tile([P, CB, 512], bf16, tag="em", name="em",
                                    bufs=2)
                    nc.vector.tensor_tensor(out=em, in0=q5c[:, t, :, :],
                                            in1=k5c[:, s, :, :], op=MULT)
                    ts_ = 5 * t + s
                    first = pair_idx[0] == 0
                    last = pair_idx[0] == len(pair_seq) - 1
                    for kc in range(CB):
                        nc.tensor.matmul(
                            ps_sc[0:25, :], sel25[:, ts_, :],
                            em[:, kc, :],
                            start=(first and kc == 0),
                            stop=(last and kc == 3))
                    pair_idx[0] += 1

            for f in range(3):
                gnt_make(f, rg3, lhsT=bc16f[0:3 * G, f, :])
            for t in range(3):
                tconv(t)
                em_pairs(t)
            gsum2 = stat.tile([2 * G, 2], f32, tag="gsum2", name="gsum2",
                              bufs=1)
            nc.sync.dma_start(out=gsum2[:], in_=bnc_out2[:])
            rg2 = affine_rg(gsum2, 2 * G)
            for f in (3, 4):
                gnt_make(f, rg2, lhsT=bc16f[0:2 * G, f, :])
                tconv(f)
                em_pairs(f)
            exp_sb = tempo.tile([32, 512], bf16, tag="exp_sb", name="exp_sb")
            nc.vector.memset(exp_sb, 0.0)
            nc.scalar.activation(out=exp_sb[0:25, :], in_=ps_sc[0:25, :],
                                 func=AF.Exp, scale=SCALE / 16.0)
            ps_d5 = psS.tile([P, 512], f32, tag="sm", name="ps_d5")
            nc.tensor.matmul(ps_d5[0:5, :], ind5, exp_sb, start=True,
                             stop=True)
            rden5 = tempo.tile([16, 512], bf16, tag="rden5", name="rden5")
            nc.vector.memset(rden5, 0.0)
            nc.vector.reciprocal(rden5[0:5, :], ps_d5[0:5, :])
            a5P = tempo.tile([P, QB, 32], bf16, tag="a5P", name="a5P")
            nc.sync.dma_start(out=a5P, in_=exp_sb, transpose=True)
            rdenP = tempo.tile([P, QB, 16], bf16, tag="rdenP", name="rdenP")
            nc.sync.dma_start(out=rdenP, in_=rden5, transpose=True)
            a5n = tempo.tile([P, QB, 32], bf16, tag="a5n", name="a5n")
            nc.vector.memset(a5n, 0.0)
            nc.vector.tensor_tensor(
                out=a5n[:, :, 0:25].rearrange("p q (t s) -> p q t s", t=5),
                in0=a5P[:, :, 0:25].rearrange("p q (t s) -> p q t s", t=5),
                in1=rdenP[:, :, 0:5].unsqueeze(3).to_broadcast([P, QB, 5, 5]),
                op=MULT)
            a_n2 = tempo.tile([P, P], bf16, tag="a_n2", name="a_n2")
            nc.sync.dma_start(out=a_n2,
                              in_=a5n.rearrange("p q w -> p (q w)"),
                              transpose=True)
            a_dr = dram.tile([32, QB, P], bf16, tag="a_dr", name="a_dr")
            nc.sync.dma_start(
                out=a_dr[:].rearrange("w q c -> q w c"),
                in_=a_n2)

            # ---- htp + wot + residual out, frame by frame ----
            xh0 = tempo.tile([P, CB, HALF], f32, tag="xh", name="xh0", bufs=2)
            nc.sync.dma_start(
                out=xh0, in_=x_d[0][:, 0:HALF].rearrange("(p j) hw -> p j hw",
                                                         p=P))
            xhs = [xh0, None, None, None, None]

            def bc_load(t):
                row = []
                for s in range(T):
                    ab = tempo.tile([P, 512], bf16, tag="a_bc", name="a_bc",
                                    bufs=8)
                    nc.sync.dma_start(
                        out=ab.rearrange("p (q c) -> p q c", q=QB),
                        in_=a_dr[5 * t + s, :, :].unsqueeze(0)
                        .to_broadcast([P, QB, P]))
                    row.append(ab)
                return row

            abc = [None] * T
            abc[0] = bc_load(0)
            for t in range(T):
                if t + 1 < T:
                    xh = tempo.tile([P, CB, HALF], f32, tag="xh",
                                    name=f"xh{t + 1}", bufs=2)
                    nc.sync.dma_start(
                        out=xh,
                        in_=x_d[t + 1][:, 0:HALF].rearrange(
                            "(p j) hw -> p j hw", p=P))
                    xhs[t + 1] = xh
                    abc[t + 1] = bc_load(t + 1)
                u = []
                for s in range(T):
                    us = tempo.tile([P, CB, 512], bf16, tag="u", name="u",
                                    bufs=5)
                    nc.vector.tensor_tensor(
                        out=us, in0=v5c[:, s, :, :],
                        in1=abc[t][s].unsqueeze(1).to_broadcast([P, CB, 512]),
                        op=MULT)
                    u.append(us)
                w01 = tempo.tile([P, CB, 512], bf16, tag="w01", name="w01",
                                 bufs=2)
                nc.vector.tensor_tensor(out=w01, in0=u[0], in1=u[1], op=ADD)
                w23 = tempo.tile([P, CB, 512], bf16, tag="w23", name="w23",
                                 bufs=2)
                nc.vector.tensor_tensor(out=w23, in0=u[2], in1=u[3], op=ADD)
                nc.vector.tensor_tensor(out=w01, in0=w01, in1=w23, op=ADD)
                htpT = tempo.tile([P, CB, HALF], bf16, tag="htpT",
                                  name="htpT", bufs=2)
                nc.vector.tensor_tensor(out=htpT, in0=w01, in1=u[4], op=ADD)
                htp8 = tempo.tile([P, CB, HALF], fp8, tag="htp8",
                                  name="htp8", bufs=2)
                nc.scalar.activation(out=htp8, in_=htpT, func=AF.Copy,
                                     scale=4.0)
                for cp in (0, 2):
                    ps = psA.tile([P, 2, 512], f32, tag="big", name="psw")
                    for i in (0, 1):
                        for kcp in (0, 1):
                            nc.tensor.matmul(
                                ps[:, i, :],
                                wot_sb[:, 2 * kcp:2 * kcp + 2,
                                       (cp + i) * P:(cp + i + 1) * P],
                                htp8[:, 2 * kcp:2 * kcp + 2, :],
                                start=(kcp == 0), stop=(kcp == 1),
                                perf_mode=DR)
                    o1 = tempo.tile([P, 2, 512], bf16, tag="o1", name="o1",
                                    bufs=2)
                    nc.scalar.activation(out=o1, in_=ps, func=AF.Copy,
                                         scale=1.0 / 256.0)
                    oc = tempo.tile([P, 2, 512], f32, tag="oc", name="oc",
                                    bufs=2)
                    nc.gpsimd.tensor_tensor(out=oc, in0=o1,
                                            in1=xhs[t][:, cp:cp + 2, :],
                                            op=ADD)
                    nc.sync.dma_start(
                        out=out_d[t].rearrange("(p j) hw -> p j hw",
                                               p=P)[:, cp:cp + 2, :],
                        in_=oc)
            tempo_cm.__exit__(None, None, None)

    nc.compile()
    return nc


# storage column s holds natural channel 4*(s % 128) + s // 128
_COL_PERM = np.array([4 * (s % P) + s // P for s in range(C)])


def _prepare_in_maps(inputs):
    import ml_dtypes
    fp8 = ml_dtypes.float8_e4m3
    x = np.asarray(inputs["x"], np.float32).reshape(B * T, C, HW)
    sel4 = np.zeros((P, G), np.float32)
    for p in range(P):
        sel4[p, p // 4] = 1.0
    bcast16 = sel4.T.copy() * 16.0
    common = {}
    for nm in ["wq", "wk", "wv", "wo", "wqt", "wkt", "wvt"]:
        w = np.asarray(inputs[nm], np.float32)   # [out, in]
        wt = np.ascontiguousarray(w.T[:, _COL_PERM]) * 16.0
        common[nm + "T"] = np.clip(wt, -240.0, 240.0).astype(fp8)
    wot = np.asarray(inputs["wot"], np.float32)
    common["wotT"] = np.clip(
        np.ascontiguousarray(wot.T[:, _COL_PERM]) * 16.0,
        -240.0, 240.0).astype(fp8)
    common["gamma_s"] = np.asarray(inputs["gamma_s"], np.float32)
    common["beta_s16"] = 16.0 * np.asarray(inputs["beta_s"], np.float32)
    common["gamma_t"] = np.asarray(inputs["gamma_t"], np.float32)
    common["beta_t16"] = 16.0 * np.asarray(inputs["beta_t"], np.float32)
    common["sel4"] = sel4
    common["bcast16"] = bcast16
    bc16f = np.zeros((P, 5, P), np.float32)
    base = [0, G, 2 * G, 0, G]
    for f in range(5):
        for p in range(P):
            bc16f[base[f] + p // 4, f, p] = 16.0
    common["bcast16f"] = bc16f
    ind5 = np.zeros((32, 5), np.float32)
    for t in range(5):
        for s in range(5):
            ind5[5 * t + s, t] = 1.0
    common["ind5"] = ind5.astype(ml_dtypes.bfloat16)
    sel25 = np.zeros((P, 25, 25), np.float32)
    for ts_ in range(25):
        sel25[:, ts_, ts_] = 1.0
    common["sel25"] = sel25.astype(ml_dtypes.bfloat16)

    in_maps = []
    for v in range(B):
        xv = x[v * T:(v + 1) * T]
        for h in range(2):
            if h == 0:
                xc = xv
            else:
                xc = np.concatenate([xv[..., HALF:], xv[..., :HALF]], axis=-1)
            m = dict(common)
            m["x"] = np.ascontiguousarray(xc)
            in_maps.append(m)
    return in_maps


def _run(inputs, trace=False):
    from concourse import bass_utils
    if "nc" not in _CACHE:
        _CACHE["nc"] = _build()
    nc = _CACHE["nc"]
    in_maps = _prepare_in_maps(inputs)
    if trace:
        try:
            from antenv.axon_hooks import get_axon_ntff_profile_hook  # noqa: F401
        except ModuleNotFoundError:
            trace = False
    res = bass_utils.run_bass_kernel_spmd(nc, in_maps, core_ids=list(range(8)),
                                          trace=trace)
    out = np.empty((B * T, C, HW), np.float32)
    for v in range(B):
        for h in range(2):
            o = res.results[2 * v + h]["out"]  # [T, C, HALF]
            if h == 0:
                out[v * T:(v + 1) * T, :, :HALF] = o
            else:
                out[v * T:(v + 1) * T, :, HALF:] = o
    return out.reshape(B * T, C, 32, 32), res


def kernel(**inputs) -> np.ndarray:
    out, _ = _run(inputs, trace=False)
    return out


# revision 49
# speedup vs baseline: 1.0639x; 1.0361x over previous
"""Trainium2 Bass kernel for nn_AttnBlock_Spatio_Temporal (B=4,T=5,C=512,H=W=32).

Distribution: 8 cores = (video b in 0..3) x (pixel-half h in 0..1); host rolls
the HW axis per core so its own 512 pixels come first. All heavy matmuls run
in fp8e4 DoubleRow (K=256 per MM). Spatial attention computes scores
TRANSPOSED (keys on partitions) so no attention transpose is needed; softmax
normalization is deferred through the v- and wo-matmuls and applied once per
output pixel (PE broadcast of 1/den). GroupNorm_t cross-half stats use two
batched pair-AllReduces (frames 0-3 in one [128,2] op, frame 4 in [32,2]).

Channel-major layout: channel c lives at (partition p, block j), c = 4p + j.
Weight matrices are host-permuted (columns) and pre-scaled x16 into fp8.
"""
import numpy as np

B, T, C, HW = 4, 5, 512, 1024
G = 32
EPS = 1e-6
P = 128
CB = C // P          # 4 channel blocks
HALF = HW // 2       # 512 own pixels
KB = HW // P         # 8 key-pixel blocks
QB = HALF // P       # 4 query/pixel blocks
SCALE = float(C) ** -0.5
INV_CNT = 1.0 / 16384.0   # per-group element count (16ch * 1024px)

_CACHE = {}


def _build():
    import concourse.bacc as bacc
    import concourse.tile as tile
    import concourse.mybir as mybir

    f32 = mybir.dt.float32
    bf16 = mybir.dt.bfloat16
    fp8 = mybir.dt.float8e4
    MULT = mybir.AluOpType.mult
    ADD = mybir.AluOpType.add
    SUB = mybir.AluOpType.subtract
    AF = mybir.ActivationFunctionType
    AX = mybir.AxisListType
    DR = mybir.MatmulPerfMode.DoubleRow

    nc = bacc.Bacc("TRN2", target_bir_lowering=False, debug=False, num_devices=8)

    x_d = nc.dram_tensor("x", [T, C, HW], f32, kind="ExternalInput").ap()
    fp8_w = ["wq", "wk", "wv", "wo", "wqt", "wkt", "wvt"]
    w_d = {nm: nc.dram_tensor(nm + "T", [C, C], fp8, kind="ExternalInput").ap()
           for nm in fp8_w}
    wot_d = nc.dram_tensor("wotT", [C, C], fp8, kind="ExternalInput").ap()
    g_d = {nm: nc.dram_tensor(nm, [C], f32, kind="ExternalInput").ap()
           for nm in ["gamma_s", "beta_s16", "gamma_t", "beta_t16"]}
    sel_d = nc.dram_tensor("sel4", [P, G], f32, kind="ExternalInput").ap()
    bc16_d = nc.dram_tensor("bcast16", [G, P], f32, kind="ExternalInput").ap()
    bc16f_d = nc.dram_tensor("bcast16f", [P, 5, P], f32,
                             kind="ExternalInput").ap()
    ind5_d = nc.dram_tensor("ind5", [32, 5], bf16, kind="ExternalInput").ap()
    sel25_d = nc.dram_tensor("sel25", [P, 25, 25], bf16,
                             kind="ExternalInput").ap()
    out_d = nc.dram_tensor("out", [T, C, HALF], f32, kind="ExternalOutput").ap()

    def cpart(ap_1d):  # [C] dram -> [128, CB] tile order (c = 4p + j)
        return ap_1d.rearrange("(p j) -> p j", p=P)

    with tile.TileContext(nc) as tc, \
         nc.allow_low_precision("fp8/bf16 pipeline by design"):
        with tc.tile_pool(name="consts", bufs=1) as consts, \
             tc.tile_pool(name="stat", bufs=2) as stat, \
             tc.tile_pool(name="spatio_p", bufs=T) as spatio_p, \
             tc.tile_pool(name="gnt_p", bufs=T) as gnt_p, \
             tc.tile_pool(name="psA", bufs=3, space="PSUM") as psA, \
             tc.tile_pool(name="psS", bufs=2, space="PSUM") as psS, \
             tc.tile_pool(name="dram", bufs=4, space="DRAM") as dram:

            # ---------------- constants ----------------
            spat_cm = tc.tile_pool(name="spat", bufs=1)
            spat = spat_cm.__enter__()
            nc.sync.dma_start(
                out=(xf0 := spat.tile([P, CB, HW], f32, tag="xf", name="xf0",
                                      bufs=2)),
                in_=x_d[0].rearrange("(p j) hw -> p j hw", p=P))
            w_sb = {}
            for nm in ["wk", "wq", "wv"]:
                w_sb[nm] = consts.tile([P, CB, C], fp8, tag="w_" + nm,
                                       name="w_" + nm)
                nc.sync.dma_start(
                    out=w_sb[nm],
                    in_=w_d[nm].rearrange("(p kc) co -> p kc co", p=P))
            gam_sb = {}
            for nm in ["gamma_s", "beta_s16", "gamma_t", "beta_t16"]:
                gam_sb[nm] = consts.tile([P, CB], f32, tag="g_" + nm,
                                         name="g_" + nm)
                nc.sync.dma_start(out=gam_sb[nm], in_=cpart(g_d[nm]))
            sel4 = consts.tile([P, G], f32, tag="sel4", name="sel4")
            nc.sync.dma_start(out=sel4, in_=sel_d)
            bc16 = consts.tile([G, P], f32, tag="bc16", name="bc16")
            nc.sync.dma_start(out=bc16, in_=bc16_d)
            bc16f = consts.tile([P, 5, P], f32, tag="bc16f", name="bc16f")
            nc.sync.dma_start(out=bc16f, in_=bc16f_d)
            eps128 = consts.tile([P, 1], f32, tag="eps", name="eps128")
            nc.vector.memset(eps128, EPS)
            ones1f = consts.tile([P, 1], fp8, tag="ones1f", name="ones1f")
            nc.vector.memset(ones1f, 1.0)
            ind5 = consts.tile([32, 5], bf16, tag="ind5", name="ind5")
            nc.sync.dma_start(out=ind5, in_=ind5_d)
            sel25 = consts.tile([P, 25, 25], bf16, tag="sel25", name="sel25")
            nc.sync.dma_start(out=sel25, in_=sel25_d)

            # ---------- GroupNorm helpers ----------
            def affine_rg(g2, rows):
                """g2: [rows,2] SBUF (sum,sumsq) -> rg [rows,2] =
                (rstd, -mu*rstd)."""
                m2 = stat.tile([P, 2], f32, tag="m2", name="m2")[0:rows, :]
                nc.scalar.activation(out=m2, in_=g2, func=AF.Copy,
                                     scale=INV_CNT)
                rg = stat.tile([P, 2], f32, tag="rg", name="rg")[0:rows, :]
                nc.vector.tensor_tensor(out=rg[:, 0:1], in0=m2[:, 0:1],
                                        in1=m2[:, 0:1], op=MULT)
                nc.vector.tensor_tensor(out=rg[:, 0:1], in0=m2[:, 1:2],
                                        in1=rg[:, 0:1], op=SUB)
                nc.scalar.activation(out=rg[:, 0:1], in_=rg[:, 0:1],
                                     func=AF.Sqrt, bias=eps128[0:rows, :],
                                     scale=1.0)
                nc.vector.reciprocal(rg[:, 0:1], rg[:, 0:1])
                nc.vector.tensor_scalar(out=rg[:, 1:2], in0=m2[:, 0:1],
                                        scalar1=rg[:, 0:1], scalar2=-1.0,
                                        op0=MULT, op1=MULT)
                return rg

            def affine_apply(rg32, gamma, beta16, tag, lhsT=None):
                """rg32: [rows,2] at base partition 0 -> scale/shift [P, CB]."""
                ps_bc = psS.tile([P, 512], f32, tag="sm", name="psbc_" + tag)
                nc.tensor.matmul(ps_bc[:, 0:2], bc16 if lhsT is None else lhsT,
                                 rg32, start=True, stop=True)
                sc = stat.tile([P, CB], f32, tag="sc" + tag, name="sc" + tag)
                sh = stat.tile([P, CB], f32, tag="sh" + tag, name="sh" + tag)
                nc.vector.tensor_scalar_mul(out=sc, in0=gamma,
                                            scalar1=ps_bc[:, 0:1])
                nc.vector.scalar_tensor_tensor(out=sh, in0=gamma,
                                               scalar=ps_bc[:, 1:2],
                                               in1=beta16, op0=MULT, op1=ADD)
                return sc, sh

            def gn_stats(src, sq_dve, sums_name):
                """per-channel (sum, sumsq) of [P, CB, n] -> sums [P, CB, 2]"""
                sums = spat.tile([P, CB, 2], f32, tag="sums", name=sums_name,
                                 bufs=2)
                nc.vector.tensor_reduce(out=sums[:, :, 0:1], in_=src,
                                        axis=AX.X, op=ADD)
                n = src.shape[2]
                sq = spat.tile([P, HW], bf16, tag="sqj", name="sq_" + sums_name,
                               bufs=2)
                for j in range(CB):
                    if sq_dve:
                        nc.vector.tensor_tensor_reduce(
                            out=sq[:, 0:n], in0=src[:, j, :], in1=src[:, j, :],
                            scale=1.0, scalar=0.0, op0=MULT, op1=ADD,
                            accum_out=sums[:, j, 1:2])
                    else:
                        nc.scalar.activation(out=sq[:, 0:n], in_=src[:, j, :],
                                             func=AF.Square,
                                             accum_out=sums[:, j, 1:2])
                return sums

            def group_sums(sums, out_rows):
                """sums [P,CB,2] -> write [32,2] group sums into out_rows."""
                ps_g = psS.tile([P, 512], f32, tag="sm", name="psg")
                nc.tensor.matmul(ps_g[0:G, 0:2 * CB], sel4,
                                 sums.rearrange("p j s -> p (j s)"),
                                 start=True, stop=True)
                nc.vector.tensor_reduce(
                    out=out_rows,
                    in_=ps_g[0:G, 0:2 * CB].rearrange("g (j s) -> g s j", s=2),
                    axis=AX.X, op=ADD)

            xfs = [None] * T
            xfs[0] = xf0
            hns = [None] * T
            spatio_tiles = [None] * T
            gnt = [None] * T
            g2b3 = stat.tile([3 * G, 2], f32, tag="g2b3", name="g2b3", bufs=1)
            g2b2 = stat.tile([2 * G, 2], f32, tag="g2b2", name="g2b2", bufs=1)

            def gn_s(f):
                sums = gn_stats(xfs[f], False, f"sums_s{f}")
                g2s = stat.tile([G, 2], f32, tag="g2s", name=f"g2s{f}")
                group_sums(sums, g2s)
                rg = affine_rg(g2s, G)
                return affine_apply(rg, gam_sb["gamma_s"], gam_sb["beta_s16"],
                                    "s")

            def hn_make(f, sc, sh):
                hn = spat.tile([P, CB, HW], fp8, tag="hn", name=f"hn{f}",
                               bufs=2)
                for j in range(CB):
                    nc.scalar.activation(out=hn[:, j, :], in_=xfs[f][:, j, :],
                                         func=AF.Identity,
                                         scale=sc[:, j:j + 1],
                                         bias=sh[:, j:j + 1])
                hns[f] = hn

            def load_x(f):
                xf = spat.tile([P, CB, HW], f32, tag="xf", name=f"xf{f}",
                               bufs=2)
                nc.sync.dma_start(
                    out=xf, in_=x_d[f].rearrange("(p j) hw -> p j hw", p=P))
                xfs[f] = xf

            # ================= spatial phase =================
            sc_s, sh_s = gn_s(0)
            for nm in ["wo", "wqt", "wkt", "wvt"]:
                w_sb[nm] = consts.tile([P, CB, C], fp8, tag="w_" + nm,
                                       name="w_" + nm)
                nc.sync.dma_start(
                    out=w_sb[nm],
                    in_=w_d[nm].rearrange("(p kc) co -> p kc co", p=P))
            wot_sb = consts.tile([P, CB, C], fp8, tag="w_wot", name="w_wot")
            nc.sync.dma_start(
                out=wot_sb, in_=wot_d.rearrange("(p kc) co -> p kc co", p=P))
            hn_make(0, sc_s, sh_s)

            for f in range(T):
                hn = hns[f]
                if f + 1 < T:
                    load_x(f + 1)

                # ---- k conv: [P, CB, HW] fp8 = 4*k ----
                k_sb = spat.tile([P, CB, HW], fp8, tag="k_sb", name="k_sb")
                for jp in (0, 2):
                    for h in (0, 1):
                        ps = psA.tile([P, 2, 512], f32, tag="big", name="psk")
                        for i in (0, 1):
                            for kcp in (0, 1):
                                nc.tensor.matmul(
                                    ps[:, i, :],
                                    w_sb["wk"][:, 2 * kcp:2 * kcp + 2,
                                               (jp + i) * P:(jp + i + 1) * P],
                                    hn[:, 2 * kcp:2 * kcp + 2,
                                       h * 512:(h + 1) * 512],
                                    start=(kcp == 0), stop=(kcp == 1),
                                    perf_mode=DR)
                        nc.scalar.activation(
                            out=k_sb[:, jp:jp + 2, h * 512:(h + 1) * 512],
                            in_=ps, func=AF.Copy, scale=1.0 / 64.0)

                # ---- q conv: [P, CB, HALF] fp8 = 4*q ----
                q_sb = spat.tile([P, CB, HALF], fp8, tag="q_sb", name="q_sb")
                for jp in (0, 2):
                    ps = psA.tile([P, 2, 512], f32, tag="big", name="psq")
                    for i in (0, 1):
                        for kcp in (0, 1):
                            nc.tensor.matmul(
                                ps[:, i, :],
                                w_sb["wq"][:, 2 * kcp:2 * kcp + 2,
                                           (jp + i) * P:(jp + i + 1) * P],
                                hn[:, 2 * kcp:2 * kcp + 2, 0:HALF],
                                start=(kcp == 0), stop=(kcp == 1),
                                perf_mode=DR)
                    nc.scalar.activation(out=q_sb[:, jp:jp + 2, :], in_=ps,
                                         func=AF.Copy, scale=1.0 / 64.0)

                # stats for next frame overlap the conv matmuls
                if f + 1 < T:
                    sc_s, sh_s = gn_s(f + 1)

                # ---- vT conv: [P(kpix), KB, C] fp8 = 4*v ----
                vT = spat.tile([P, KB, C], fp8, tag="vT", name="vT")
                for pp in (0, 2, 4, 6):
                    ps = psA.tile([P, 2, 512], f32, tag="big", name="psv")
                    for i in (0, 1):
                        for kcp in (0, 1):
                            nc.tensor.matmul(
                                ps[:, i, :],
                                hn[:, 2 * kcp:2 * kcp + 2,
                                   (pp + i) * P:(pp + i + 1) * P],
                                w_sb["wv"][:, 2 * kcp:2 * kcp + 2, :],
                                start=(kcp == 0), stop=(kcp == 1),
                                perf_mode=DR)
                    nc.vector.tensor_scalar(out=vT[:, pp:pp + 2, :], in0=ps,
                                            scalar1=1.0 / 64.0, scalar2=0.0,
                                            op0=MULT, op1=ADD)

                if f + 1 < T:
                    hn_make(f + 1, sc_s, sh_s)

                # ---- scoresT + exp: expT [P(kpix), KB, HALF] fp8 ----
                expT = spat.tile([P, KB, HALF], fp8, tag="expT", name="expT")
                for kp in (0, 2, 4, 6):
                    ps = psA.tile([P, 2, 512], f32, tag="big", name="pssc")
                    for i in (0, 1):
                        for kcp in (0, 1):
                            nc.tensor.matmul(
                                ps[:, i, :],
                                k_sb[:, 2 * kcp:2 * kcp + 2,
                                     (kp + i) * P:(kp + i + 1) * P],
                                q_sb[:, 2 * kcp:2 * kcp + 2, :],
                                start=(kcp == 0), stop=(kcp == 1),
                                perf_mode=DR)
                    nc.scalar.activation(out=expT[:, kp:kp + 2, :], in_=ps,
                                         func=AF.Exp, scale=SCALE / 16.0)

                # ---- den = sum_k exp ; rdenB = 2/den bcast ----
                ps_den = psS.tile([P, 512], f32, tag="sm", name="psden")
                for i in range(KB):
                    nc.tensor.matmul(ps_den[0:1, :], ones1f,
                                     expT[:, i, :],
                                     start=(i == 0), stop=(i == KB - 1))
                rden = spat.tile([1, 512], bf16, tag="rden", name="rden")
                nc.vector.reciprocal(rden, ps_den[0:1, :])
                rden_dr = dram.tile([512], bf16, tag="rden_dr",
                                    name="rden_dr")
                nc.sync.dma_start(out=rden_dr[:], in_=rden)
                rdenB = spat.tile([P, 512], bf16, tag="rdenB", name="rdenB")
                nc.sync.dma_start(
                    out=rdenB,
                    in_=rden_dr[:].unsqueeze(0).to_broadcast([P, 512]))

                # ---- hsp = vT'.T @ expT : [P, CB, HALF] fp8 = hsp_u/32 ----
                hsp = spat.tile([P, CB, HALF], fp8, tag="hsp", name="hsp")
                for cp in (0, 2):
                    ps = psA.tile([P, 2, 512], f32, tag="big", name="psh")
                    for i in (0, 1):
                        for kbp in range(4):
                            nc.tensor.matmul(
                                ps[:, i, :],
                                vT[:, 2 * kbp:2 * kbp + 2,
                                   (cp + i) * P:(cp + i + 1) * P],
                                expT[:, 2 * kbp:2 * kbp + 2, :],
                                start=(kbp == 0), stop=(kbp == 3),
                                perf_mode=DR)
                    nc.vector.tensor_scalar(out=hsp[:, cp:cp + 2, :], in0=ps,
                                            scalar1=1.0 / 128.0, scalar2=0.0,
                                            op0=MULT, op1=ADD)

                # ---- wo conv + deferred normalize + residual ----
                spatio = spatio_p.tile([P, CB, HALF], bf16, tag="spatio",
                                       name=f"spatio{f}")
                for cp in (0, 2):
                    ps = psA.tile([P, 2, 512], f32, tag="big", name="pso")
                    for i in (0, 1):
                        for kcp in (0, 1):
                            nc.tensor.matmul(
                                ps[:, i, :],
                                w_sb["wo"][:, 2 * kcp:2 * kcp + 2,
                                           (cp + i) * P:(cp + i + 1) * P],
                                hsp[:, 2 * kcp:2 * kcp + 2, :],
                                start=(kcp == 0), stop=(kcp == 1),
                                perf_mode=DR)
                    s_n = spat.tile([P, 2, 512], bf16, tag="s_n", name="s_n",
                                    bufs=2)
                    nc.vector.tensor_tensor(
                        out=s_n, in0=ps,
                        in1=rdenB.unsqueeze(1).to_broadcast([P, 2, 512]),
                        op=MULT)
                    nc.gpsimd.tensor_tensor(out=spatio[:, cp:cp + 2, :],
                                            in0=s_n,
                                            in1=xfs[f][:, cp:cp + 2, 0:512],
                                            op=ADD)

                spatio_tiles[f] = spatio

                # ---- GN_t partial stats ----
                sums_t = gn_stats(spatio, False, f"sums_t{f}")
                if f < 3:
                    group_sums(sums_t, g2b3[f * G:(f + 1) * G, :])
                else:
                    group_sums(sums_t, g2b2[(f - 3) * G:(f - 2) * G, :])

                if f == 2:
                    bnc_in3 = dram.tile([3 * G, 2], f32, tag="bnc_in3",
                                        name="bnc_in3")
                    bnc_out3 = dram.tile([3 * G, 2], f32, tag="bnc_out3",
                                         name="bnc_out3")
                    nc.sync.dma_start(out=bnc_in3[:], in_=g2b3[:])
                    nc.gpsimd.collective_compute(
                        "AllReduce", ADD,
                        replica_groups=[[0, 1], [2, 3], [4, 5], [6, 7]],
                        ins=[bnc_in3.opt()], outs=[bnc_out3.opt()])

            # frames 3-4 collective
            bnc_in2 = dram.tile([2 * G, 2], f32, tag="bnc_in2", name="bnc_in2")
            bnc_out2 = dram.tile([2 * G, 2], f32, tag="bnc_out2",
                                 name="bnc_out2")
            nc.sync.dma_start(out=bnc_in2[:], in_=g2b2[:])
            nc.gpsimd.collective_compute(
                "AllReduce", ADD,
                replica_groups=[[0, 1], [2, 3], [4, 5], [6, 7]],
                ins=[bnc_in2.opt()], outs=[bnc_out2.opt()])

            # ---- temporal GN affines (frames 0-2 batched) + gnt ----
            spat_cm.__exit__(None, None, None)
            tempo_cm = tc.tile_pool(name="tempo", bufs=1)
            tempo = tempo_cm.__enter__()
            gsum3 = stat.tile([3 * G, 2], f32, tag="gsum3", name="gsum3",
                              bufs=1)
            nc.sync.dma_start(out=gsum3[:], in_=bnc_out3[:])
            rg3 = affine_rg(gsum3, 3 * G)

            def gnt_make(f, rg32, lhsT=None):
                sc, sh = affine_apply(rg32, gam_sb["gamma_t"],
                                      gam_sb["beta_t16"], "t", lhsT=lhsT)
                g = gnt_p.tile([P, CB, HALF], fp8, tag="gnt", name=f"gnt{f}")
                for j in range(CB):
                    nc.vector.tensor_scalar(out=g[:, j, :],
                                            in0=spatio_tiles[f][:, j, :],
                                            scalar1=sc[:, j:j + 1],
                                            scalar2=sh[:, j:j + 1],
                                            op0=MULT, op1=ADD)
                gnt[f] = g

            # ================= temporal phase =================
            q5c = tempo.tile([P, T, CB, 512], bf16, tag="q5c", name="q5c")
            k5c = tempo.tile([P, T, CB, 512], bf16, tag="k5c", name="k5c")
            v5c = tempo.tile([P, T, CB, 512], bf16, tag="v5c", name="v5c")

            def tconv(t):
                for nm, dst in (("wqt", q5c), ("wkt", k5c), ("wvt", v5c)):
                    for jp in (0, 2):
                        ps = psA.tile([P, 2, 512], f32, tag="big", name="pst")
                        for i in (0, 1):
                            for kcp in (0, 1):
                                nc.tensor.matmul(
                                    ps[:, i, :],
                                    w_sb[nm][:, 2 * kcp:2 * kcp + 2,
                                             (jp + i) * P:(jp + i + 1) * P],
                                    gnt[t][:, 2 * kcp:2 * kcp + 2, :],
                                    start=(kcp == 0), stop=(kcp == 1),
                                    perf_mode=DR)
                        nc.scalar.activation(out=dst[:, t, jp:jp + 2, :],
                                             in_=ps, func=AF.Copy,
                                             scale=1.0 / 64.0)

            ps_sc = psS.tile([P, 512], f32, tag="sm", name="ps_sc")
            pair_seq = sorted([(t, s) for t in range(T) for s in range(T)],
                              key=lambda p: max(p))
            pair_idx = [0]

            def em_pairs(upto):
                while pair_idx[0] < len(pair_seq):
                    t, s = pair_seq[pair_idx[0]]
                    if max(t, s) > upto:
                        return
                    em = tempo.tile([P, CB, 512], bf16, tag="em", name="em",
                                    bufs=2)
                    nc.vector.tensor_tensor(out=em, in0=q5c[:, t, :, :],
                                            in1=k5c[:, s, :, :], op=MULT)
                    ts_ = 5 * t + s
                    first = pair_idx[0] == 0
                    last = pair_idx[0] == len(pair_seq) - 1
                    for kc in range(CB):
                        nc.tensor.matmul(
                            ps_sc[0:25, :], sel25[:, ts_, :],
                            em[:, kc, :],
                            start=(first and kc == 0),
                            stop=(last and kc == 3))
                    pair_idx[0] += 1

            for f in range(3):
                gnt_make(f, rg3, lhsT=bc16f[0:3 * G, f, :])
            for t in range(3):
                tconv(t)
                em_pairs(t)
            gsum2 = stat.tile([2 * G, 2], f32, tag="gsum2", name="gsum2",
                              bufs=1)
            nc.sync.dma_start(out=gsum2[:], in_=bnc_out2[:])
            rg2 = affine_rg(gsum2, 2 * G)
            for f in (3, 4):
                gnt_make(f, rg2, lhsT=bc16f[0:2 * G, f, :])
                tconv(f)
                em_pairs(f)
            exp_sb = tempo.tile([32, 512], bf16, tag="exp_sb", name="exp_sb")
            nc.vector.memset(exp_sb, 0.0)
            nc.scalar.activation(out=exp_sb[0:25, :], in_=ps_sc[0:25, :],
                                 func=AF.Exp, scale=SCALE / 16.0)
            ps_d5 = psS.tile([P, 512], f32, tag="sm", name="ps_d5")
            nc.tensor.matmul(ps_d5[0:5, :], ind5, exp_sb, start=True,
                             stop=True)
            rden5 = tempo.tile([16, 512], bf16, tag="rden5", name="rden5")
            nc.vector.memset(rden5, 0.0)
            nc.vector.reciprocal(rden5[0:5, :], ps_d5[0:5, :])
            a5P = tempo.tile([P, QB, 32], bf16, tag="a5P", name="a5P")
            nc.sync.dma_start(out=a5P, in_=exp_sb, transpose=True)
            rdenP = tempo.tile([P, QB, 16], bf16, tag="rdenP", name="rdenP")
            nc.sync.dma_start(out=rdenP, in_=rden5, transpose=True)
            a5n = tempo.tile([P, QB, 32], bf16, tag="a5n", name="a5n")
            nc.vector.memset(a5n, 0.0)
            nc.vector.tensor_tensor(
                out=a5n[:, :, 0:25].rearrange("p q (t s) -> p q t s", t=5),
                in0=a5P[:, :, 0:25].rearrange("p q (t s) -> p q t s", t=5),
                in1=rdenP[:, :, 0:5].unsqueeze(3).to_broadcast([P, QB, 5, 5]),
                op=MULT)
            a_n2 = tempo.tile([P, P], bf16, tag="a_n2", name="a_n2")
            nc.sync.dma_start(out=a_n2,
                              in_=a5n.rearrange("p q w -> p (q w)"),
                              transpose=True)
            a_dr = dram.tile([32, QB, P], bf16, tag="a_dr", name="a_dr")
            nc.sync.dma_start(
                out=a_dr[:].rearrange("w q c -> q w c"),
                in_=a_n2)

            # ---- htp + wot + residual out, frame by frame ----
            xh0 = tempo.tile([P, CB, HALF], f32, tag="xh", name="xh0", bufs=2)
            nc.sync.dma_start(
                out=xh0, in_=x_d[0][:, 0:HALF].rearrange("(p j) hw -> p j hw",
                                                         p=P))
            xhs = [xh0, None, None, None, None]

            def bc_load(t):
                row = []
                for s in range(T):
                    ab = tempo.tile([P, 512], bf16, tag="a_bc", name="a_bc",
                                    bufs=8)
                    nc.sync.dma_start(
                        out=ab.rearrange("p (q c) -> p q c", q=QB),
                        in_=a_dr[5 * t + s, :, :].unsqueeze(0)
                        .to_broadcast([P, QB, P]))
                    row.append(ab)
                return row

            abc = [None] * T
            abc[0] = bc_load(0)
            for t in range(T):
                if t + 1 < T:
                    xh = tempo.tile([P, CB, HALF], f32, tag="xh",
                                    name=f"xh{t + 1}", bufs=2)
                    nc.sync.dma_start(
                        out=xh,
                        in_=x_d[t + 1][:, 0:HALF].rearrange(
                            "(p j) hw -> p j hw", p=P))
                    xhs[t + 1] = xh
                    abc[t + 1] = bc_load(t + 1)
                u = []
                for s in range(T):
                    us = tempo.tile([P, CB, 512], bf16, tag="u", name="u",
                                    bufs=5)
                    nc.vector.tensor_tensor(
                        out=us, in0=v5c[:, s, :, :],
                        in1=abc[t][s].unsqueeze(1).to_broadcast([P, CB, 512]),
                        op=MULT)
                    u.append(us)
                w01 = tempo.tile([P, CB, 512], bf16, tag="w01", name="w01",
                                 bufs=2)
                nc.vector.tensor_tensor(out=w01, in0=u[0], in1=u[1], op=ADD)
                w23 = tempo.tile([P, CB, 512], bf16, tag="w23", name="w23",
                                 bufs=2)
                nc.vector.tensor_tensor(out=w23, in0=u[2], in1=u[3], op=ADD)
                nc.vector.tensor_tensor(out=w01, in0=w01, in1=w23, op=ADD)
                htpT = tempo.tile([P, CB, HALF], bf16, tag="htpT",
                                  name="htpT", bufs=2)
                nc.vector.tensor_tensor(out=htpT, in0=w01, in1=u[4], op=ADD)
                htp8 = tempo.tile([P, CB, HALF], fp8, tag="htp8",
                                  name="htp8", bufs=2)
                nc.scalar.activation(out=htp8, in_=htpT, func=AF.Copy,
                                     scale=4.0)
                for cp in (0, 2):
                    ps = psA.tile([P, 2, 512], f32, tag="big", name="psw")
                    for i in (0, 1):
                        for kcp in (0, 1):
                            nc.tensor.matmul(
                                ps[:, i, :],
                                wot_sb[:, 2 * kcp:2 * kcp + 2,
                                       (cp + i) * P:(cp + i + 1) * P],
                                htp8[:, 2 * kcp:2 * kcp + 2, :],
                                start=(kcp == 0), stop=(kcp == 1),
                                perf_mode=DR)
                    o1 = tempo.tile([P, 2, 512], bf16, tag="o1", name="o1",
                                    bufs=2)
                    nc.scalar.activation(out=o1, in_=ps, func=AF.Copy,
                                         scale=1.0 / 256.0)
                    oc = tempo.tile([P, 2, 512], f32, tag="oc", name="oc",
                                    bufs=2)
                    nc.gpsimd.tensor_tensor(out=oc, in0=o1,
                                            in1=xhs[t][:, cp:cp + 2, :],
                                            op=ADD)
                    nc.sync.dma_start(
                        out=out_d[t].rearrange("(p j) hw -> p j hw",
                                               p=P)[:, cp:cp + 2, :],
                        in_=oc)
            tempo_cm.__exit__(None, None, None)

    nc.compile()
    return nc


# storage column s holds natural channel 4*(s % 128) + s // 128
_COL_PERM = np.array([4 * (s % P) + s // P for s in range(C)])


def _prepare_in_maps(inputs):
    import ml_dtypes
    fp8 = ml_dtypes.float8_e4m3
    x = np.asarray(inputs["x"], np.float32).reshape(B * T, C, HW)
    sel4 = np.zeros((P, G), np.float32)
    for p in range(P):
        sel4[p, p // 4] = 1.0
    bcast16 = sel4.T.copy() * 16.0
    common = {}
    for nm in ["wq", "wk", "wv", "wo", "wqt", "wkt", "wvt"]:
        w = np.asarray(inputs[nm], np.float32)   # [out, in]
        wt = np.ascontiguousarray(w.T[:, _COL_PERM]) * 16.0
        common[nm + "T"] = np.clip(wt, -240.0, 240.0).astype(fp8)
    wot = np.asarray(inputs["wot"], np.float32)
    common["wotT"] = np.clip(
        np.ascontiguousarray(wot.T[:, _COL_PERM]) * 16.0,
        -240.0, 240.0).astype(fp8)
    common["gamma_s"] = np.asarray(inputs["gamma_s"], np.float32)
    common["beta_s16"] = 16.0 * np.asarray(inputs["beta_s"], np.float32)
    common["gamma_t"] = np.asarray(inputs["gamma_t"], np.float32)
    common["beta_t16"] = 16.0 * np.asarray(inputs["beta_t"], np.float32)
    common["sel4"] = sel4
    common["bcast16"] = bcast16
    bc16f = np.zeros((P, 5, P), np.float32)
    base = [0, G, 2 * G, 0, G]
    for f in range(5):
        for p in range(P):
            bc16f[base[f] + p // 4, f, p] = 16.0
    common["bcast16f"] = bc16f
    ind5 = np.zeros((32, 5), np.float32)
    for t in range(5):
        for s in range(5):
            ind5[5 * t + s, t] = 1.0
    common["ind5"] = ind5.astype(ml_dtypes.bfloat16)
    sel25 = np.zeros((P, 25, 25), np.float32)
    for ts_ in range(25):
        sel25[:, ts_, ts_] = 1.0
    common["sel25"] = sel25.astype(ml_dtypes.bfloat16)

    in_maps = []
    for v in range(B):
        xv = x[v * T:(v + 1) * T]
        for h in range(2):
            if h == 0:
                xc = xv
            else:
                xc = np.concatenate([xv[..., HALF:], xv[..., :HALF]], axis=-1)
            m = dict(common)
            m["x"] = np.ascontiguousarray(xc)
            in_maps.append(m)
    return in_maps


def _run(inputs, trace=False):
    from concourse import bass_utils
    if "nc" not in _CACHE:
        _CACHE["nc"] = _build()
    nc = _CACHE["nc"]
    in_maps = _prepare_in_maps(inputs)
    if trace:
        try:
            from antenv.axon_hooks import get_axon_ntff_profile_hook  # noqa: F401
        except ModuleNotFoundError:
            trace = False
    res = bass_utils.run_bass_kernel_spmd(nc, in_maps, core_ids=list(range(8)),
                                          trace=trace)
    out = np.empty((B * T, C, HW), np.float32)
    for v in range(B):
        for h in range(2):
            o = res.results[2 * v + h]["out"]  # [T, C, HALF]
            if h == 0:
                out[v * T:(v + 1) * T, :, :HALF] = o
            else:
                out[v * T:(v + 1) * T, :, HALF:] = o
    return out.reshape(B * T, C, 32, 32), res


def kernel(**inputs) -> np.ndarray:
    out, _ = _run(inputs, trace=False)
    return out
